# revision 30
# baseline (speedup 1.0000x reference)
"""Trainium2 Bass kernel for nn_Decoder (2-layer LSTM + 3 FC + top-k decode).

Strategy: pure data parallelism over batch (2048 -> 8 cores x 256).
Feature-major activations [feat, batch]. All matmuls are 3-term bf16
splits (hi/lo), empirically exact for every argmax decision. fc1/fc2/fc3
fold on the host into one 256->10000 matmul (fp64 compose). For steps
>= 1 the LSTM1 input matmul becomes one-hot table matmuls
(tables = W1ih @ fcq{w,l}_W, host fp64). All biases (gates and fc3) are
seeded into PSUM by tiny K=3 ones-matmuls of bf16 bias triples, so the
scalar engine runs bias-free activations and the vector engine does
plain PSUM evacuation. The decode pipeline is split into two 128-row
chunks and software-pipelined: the next step's LSTM work is emitted
between this step's per-chunk merge/one-hot phases so PE never drains.
"""
import numpy as np
import ml_dtypes

B, D, H = 2048, 256, 256
K4, QW, QL, DELTA = 4, 100, 100, 16
Q = QW * QL
NCORES = 8
BS = B // NCORES          # 256 rows per core
TW = 500                  # fc3 tile width
NT = Q // TW              # 20 tiles per chunk
G4 = 4 * H                # 1024 gates
# scan quarters (in tiles): last one small to shorten the critical tail
QTILES = (6, 6, 6, 2)
QBASE = (0, 3000, 6000, 9000)

_CACHE = {}


def _build_nc(delta=DELTA):
    import concourse.mybir as mybir
    import concourse.tile as tile
    import concourse.bacc as bacc
    from concourse.masks import make_identity

    F32 = mybir.dt.float32
    BF16 = mybir.dt.bfloat16
    U32 = mybir.dt.uint32
    AF = mybir.ActivationFunctionType
    ALU = mybir.AluOpType

    nc = bacc.Bacc(None, target_bir_lowering=False, debug=False)

    def din(name, shape, dt=F32):
        return nc.dram_tensor(name, shape, dt, kind="ExternalInput")

    # per-core inputs
    xh_in = din("xh", [2, 128, BS], BF16)
    xl_in = din("xl", [2, 128, BS], BF16)
    c1_in = din("c1_fm", [2, 128, BS])
    c2_in = din("c2_fm", [2, 128, BS])
    h1h_in = din("h1h", [2, 128, BS], BF16)
    h1l_in = din("h1l", [2, 128, BS], BF16)
    h2h_in = din("h2h", [2, 128, BS], BF16)
    h2l_in = din("h2l", [2, 128, BS], BF16)
    # shared weights (bf16 hi/lo pairs, lhsT layout)
    w1ihh_in = din("w1ihTh", [2, 128, G4], BF16)
    w1ihl_in = din("w1ihTl", [2, 128, G4], BF16)
    w1hhh_in = din("w1hhTh", [2, 128, G4], BF16)
    w1hhl_in = din("w1hhTl", [2, 128, G4], BF16)
    w2ihh_in = din("w2ihTh", [2, 128, G4], BF16)
    w2ihl_in = din("w2ihTl", [2, 128, G4], BF16)
    w2hhh_in = din("w2hhTh", [2, 128, G4], BF16)
    w2hhl_in = din("w2hhTl", [2, 128, G4], BF16)
    w3h_in = din("w3Th", [2, 128, Q], BF16)
    w3l_in = din("w3Tl", [2, 128, Q], BF16)
    awh_in = din("awTh", [100, G4], BF16)
    awl_in = din("awTl", [100, G4], BF16)
    alh_in = din("alTh", [100, G4], BF16)
    all_in = din("alTl", [100, G4], BF16)
    b1t_in = din("b1t", [3, G4], BF16)     # raw lstm1 bias triple (t=0)
    b1ft_in = din("b1ft", [3, G4], BF16)   # folded lstm1 bias triple (t>=1)
    b2t_in = din("b2t", [3, G4], BF16)
    b3t_in = din("b3t", [3, Q], BF16)

    idx_out = nc.dram_tensor("idx_out", [2, 128, 20], U32, kind="ExternalOutput")

    with tile.TileContext(nc) as tc:
        with (
            tc.tile_pool(name="wp", bufs=1) as wp,
            tc.tile_pool(name="st", bufs=1) as st,
            tc.tile_pool(name="wk", bufs=2) as wk,
            tc.tile_pool(name="p3", bufs=4, space="PSUM") as p3,
            tc.tile_pool(name="pg", bufs=2, space="PSUM") as pg,
        ):
            # ---- weight / const loads (ordered by first use) ----
            def wload(src, shape, tag, dt=F32):
                t = wp.tile(shape, dt, tag=tag, name=tag)
                if len(shape) == 3 and shape[1] == 2:
                    nc.sync.dma_start(t[:], src[:].rearrange("c p f -> p c f"))
                else:
                    nc.sync.dma_start(t[:], src[:])
                return t

            w1ihh = wload(w1ihh_in, [128, 2, G4], "w1ihh", BF16)
            w1ihl = wload(w1ihl_in, [128, 2, G4], "w1ihl", BF16)
            w1hhh = wload(w1hhh_in, [128, 2, G4], "w1hhh", BF16)
            w1hhl = wload(w1hhl_in, [128, 2, G4], "w1hhl", BF16)
            b1t = wload(b1t_in, [3, G4], "b1t", BF16)
            b2t = wload(b2t_in, [3, G4], "b2t", BF16)
            w2ihh = wload(w2ihh_in, [128, 2, G4], "w2ihh", BF16)
            w2ihl = wload(w2ihl_in, [128, 2, G4], "w2ihl", BF16)
            w2hhh = wload(w2hhh_in, [128, 2, G4], "w2hhh", BF16)
            w2hhl = wload(w2hhl_in, [128, 2, G4], "w2hhl", BF16)
            w3h = wload(w3h_in, [128, 2, Q], "w3h", BF16)
            w3l = wload(w3l_in, [128, 2, Q], "w3l", BF16)
            b3t = wload(b3t_in, [3, Q], "b3t", BF16)
            awh = wload(awh_in, [100, G4], "awh", BF16)
            awl = wload(awl_in, [100, G4], "awl", BF16)
            alh = wload(alh_in, [100, G4], "alh", BF16)
            all_ = wload(all_in, [100, G4], "all", BF16)
            b1ft = wload(b1ft_in, [3, G4], "b1ft", BF16)

            one3 = wp.tile([3, 128], BF16)
            nc.vector.memset(one3[:], 1.0)
            ident = wp.tile([128, 128], F32)
            make_identity(nc, ident[:])
            io_f = wp.tile([128, 100], F32)
            nc.gpsimd.iota(io_f[:], pattern=[[1, 100]], base=0,
                           channel_multiplier=0,
                           allow_small_or_imprecise_dtypes=True)
            io100 = wp.tile([128, 100], F32)
            nc.gpsimd.iota(io100[:], pattern=[[100, 100]], base=0,
                           channel_multiplier=0,
                           allow_small_or_imprecise_dtypes=True)
            io32 = wp.tile([128, 32], F32)
            nc.gpsimd.iota(io32[:], pattern=[[1, 32]], base=0,
                           channel_multiplier=0,
                           allow_small_or_imprecise_dtypes=True)

            # ---- persistent state ----
            def sload(src, tag, dt=F32):
                t = st.tile([128, 2, BS], dt, tag=tag, name=tag)
                nc.sync.dma_start(t[:], src[:].rearrange("c p b -> p c b"))
                return t

            xh = sload(xh_in, "xh", BF16)
            xl = sload(xl_in, "xl", BF16)
            c1_t = sload(c1_in, "c1")
            c2_t = sload(c2_in, "c2")
            h1h = sload(h1h_in, "h1h", BF16)
            h1l = sload(h1l_in, "h1l", BF16)
            h2h = sload(h2h_in, "h2h", BF16)
            h2l = sload(h2l_in, "h2l", BF16)
            h1_t = st.tile([128, 2, BS], F32, tag="h1", name="h1")
            h2_t = st.tile([128, 2, BS], F32, tag="h2", name="h2")
            ohwT = st.tile([100, BS], BF16, tag="ohwT", name="ohwT")
            ohlT = st.tile([100, BS], BF16, tag="ohlT", name="ohlT")
            outi = st.tile([128, 2, 20], U32, tag="outi", name="outi")
            nc.vector.memset(outi[:], 0)

            def bsl(bc):
                return slice(128 * bc, 128 * (bc + 1))

            # ---- per-chunk LSTM matmul phases ----
            def gates_layer1(bc, t):
                """gates1 psum for chunk bc: bias seed + x/tables + whh1."""
                gp = pg.tile([128, 8, 128], F32, tag="g1", name="g1")
                bs = bsl(bc)
                bt = b1t if t == 0 else b1ft
                for g in range(8):
                    sl = slice(128 * g, 128 * (g + 1))
                    o = gp[:, g, :]
                    nc.tensor.matmul(o, bt[:, sl], one3[:],
                                     start=True, stop=False)
                    for k in range(2):
                        nc.tensor.matmul(o, w1hhh[:, k, sl], h1h[:, k, bs],
                                         start=False, stop=False)
                        nc.tensor.matmul(o, w1hhh[:, k, sl], h1l[:, k, bs],
                                         start=False, stop=False)
                        nc.tensor.matmul(o, w1hhl[:, k, sl], h1h[:, k, bs],
                                         start=False, stop=False)
                    if t == 0:
                        for k in range(2):
                            nc.tensor.matmul(o, w1ihh[:, k, sl], xh[:, k, bs],
                                             start=False, stop=False)
                            nc.tensor.matmul(o, w1ihh[:, k, sl], xl[:, k, bs],
                                             start=False, stop=False)
                            nc.tensor.matmul(o, w1ihl[:, k, sl], xh[:, k, bs],
                                             start=False, stop=(k == 1))
                    else:
                        nc.tensor.matmul(o, awh[:, sl], ohwT[:, bs],
                                         start=False, stop=False)
                        nc.tensor.matmul(o, awl[:, sl], ohwT[:, bs],
                                         start=False, stop=False)
                        nc.tensor.matmul(o, alh[:, sl], ohlT[:, bs],
                                         start=False, stop=False)
                        nc.tensor.matmul(o, all_[:, sl], ohlT[:, bs],
                                         start=False, stop=True)
                return gp

            def gates_layer2(bc):
                gp = pg.tile([128, 8, 128], F32, tag="g1", name="g2")
                bs = bsl(bc)
                for g in range(8):
                    sl = slice(128 * g, 128 * (g + 1))
                    o = gp[:, g, :]
                    nc.tensor.matmul(o, b2t[:, sl], one3[:],
                                     start=True, stop=False)
                    for k in range(2):
                        nc.tensor.matmul(o, w2ihh[:, k, sl], h1h[:, k, bs],
                                         start=False, stop=False)
                        nc.tensor.matmul(o, w2ihh[:, k, sl], h1l[:, k, bs],
                                         start=False, stop=False)
                        nc.tensor.matmul(o, w2ihl[:, k, sl], h1h[:, k, bs],
                                         start=False, stop=False)
                    for k in range(2):
                        nc.tensor.matmul(o, w2hhh[:, k, sl], h2h[:, k, bs],
                                         start=False, stop=False)
                        nc.tensor.matmul(o, w2hhh[:, k, sl], h2l[:, k, bs],
                                         start=False, stop=False)
                        nc.tensor.matmul(o, w2hhl[:, k, sl], h2h[:, k, bs],
                                         start=False, stop=(k == 1))
                return gp

            def gate_acts(bc, gp):
                """bias-free activations over both feature chunks at once."""
                si = wk.tile([128, 2, 128], F32, tag="si")
                sf = wk.tile([128, 2, 128], F32, tag="sf")
                tg = wk.tile([128, 2, 128], F32, tag="tg")
                so = wk.tile([128, 2, 128], F32, tag="so")
                nc.scalar.activation(si[:], gp[:, 0:2, :], AF.Sigmoid)
                nc.scalar.activation(sf[:], gp[:, 2:4, :], AF.Sigmoid)
                nc.scalar.activation(tg[:], gp[:, 4:6, :], AF.Tanh)
                nc.scalar.activation(so[:], gp[:, 6:8, :], AF.Sigmoid)
                return si, sf, tg, so

            def cell_update(bc, acts, cT, hT, hh, hl):
                si, sf, tg, so = acts
                bs = bsl(bc)
                csl = cT[:, :, bs]
                hsl = hT[:, :, bs]
                t1 = wk.tile([128, 2, 128], F32, tag="t1", bufs=1)
                t2 = wk.tile([128, 2, 128], F32, tag="t2", bufs=1)
                nc.vector.tensor_mul(t1[:], sf[:], csl)
                nc.vector.tensor_mul(t2[:], si[:], tg[:])
                nc.vector.tensor_add(csl, t1[:], t2[:])
                t3 = wk.tile([128, 2, 128], F32, tag="t3", bufs=1)
                nc.scalar.activation(t3[:], csl, AF.Tanh)
                nc.vector.tensor_mul(hsl, so[:], t3[:])
                nc.vector.tensor_copy(hh[:, :, bs], hsl)
                nc.vector.tensor_sub(hl[:, :, bs], hsl, hh[:, :, bs])

            # ---- fc3 + scan for one chunk ----
            def fc3_scan(bc):
                bs = bsl(bc)
                lq = [wk.tile([128, 3000], F32, tag="logq", name="logq",
                              bufs=2) for _ in QTILES]
                cand_v = wk.tile([128, 32], F32, tag="candv", name="candv")
                cand_i = wk.tile([128, 32], F32, tag="candi", name="candi")
                stats = [(h2h[:, 0, bs], w3h[:, 0, :]),
                         (h2h[:, 1, bs], w3h[:, 1, :]),
                         (h2l[:, 0, bs], w3h[:, 0, :]),
                         (h2l[:, 1, bs], w3h[:, 1, :]),
                         (h2h[:, 0, bs], w3l[:, 0, :]),
                         (h2h[:, 1, bs], w3l[:, 1, :])]
                qt = 0
                for tt in range(NT):
                    n0 = tt * TW
                    pt = p3.tile([128, 512], F32, tag="fc3p", name="fc3p")
                    o = pt[:, 0:TW]
                    # seed PSUM with the (3-term bf16) fc3 bias, then accumulate
                    nc.tensor.matmul(o, one3[:], b3t[:, n0:n0 + TW],
                                     start=True, stop=False)
                    for j, (stat, w) in enumerate(stats):
                        nc.tensor.matmul(o, stat, w[:, n0:n0 + TW],
                                         start=False, stop=(j == 5))
                    off = n0 - QBASE[qt]
                    nc.vector.tensor_copy(lq[qt][:, off:off + TW], o)
                    if tt + 1 == (QBASE[qt] + 500 * QTILES[qt]) // TW:
                        qspan = lq[qt][:, 0:500 * QTILES[qt]]
                        m8q = wk.tile([128, 8], F32, tag="m8q", name="m8q")
                        i8q = wk.tile([128, 8], U32, tag="i8q", name="i8q")
                        nc.vector.max(m8q[:], qspan)
                        nc.vector.max_index(i8q[:], m8q[:], qspan)
                        nc.vector.tensor_copy(cand_v[:, 8 * qt:8 * qt + 8],
                                              m8q[:])
                        i8f = wk.tile([128, 8], F32, tag="i8f", name="i8f")
                        nc.vector.tensor_copy(i8f[:], i8q[:])
                        nc.vector.tensor_scalar(
                            cand_i[:, 8 * qt:8 * qt + 8], i8f[:],
                            float(QBASE[qt]), None, op0=ALU.add)
                        qt += 1
                return cand_v, cand_i

            def merge_onehot(bc, t, cand_v, cand_i):
                """top-k merge, trajectory index write, one-hot build."""
                vm8 = wk.tile([128, 8], F32, tag="vm8", name="vm8")
                pm8 = wk.tile([128, 8], U32, tag="pm8", name="pm8")
                nc.vector.max(vm8[:], cand_v[:])
                nc.vector.max_index(pm8[:], vm8[:], cand_v[:])
                pmf = wk.tile([128, 8], F32, tag="pmf", name="pmf")
                nc.vector.tensor_copy(pmf[:], pm8[:])
                nk = 4 if t == 0 else 1
                qsel = wk.tile([128, 4], F32, tag="qsel", name="qsel")
                for kk in range(nk):
                    ohp = wk.tile([128, 32], F32, tag="ohp", name="ohp")
                    nc.vector.tensor_scalar(ohp[:], io32[:], pmf[:, kk:kk + 1],
                                            None, op0=ALU.is_equal)
                    tmq = wk.tile([128, 32], F32, tag="tmq", name="tmq")
                    nc.vector.tensor_mul(tmq[:], ohp[:], cand_i[:])
                    nc.vector.tensor_reduce(qsel[:, kk:kk + 1], tmq[:],
                                            axis=mybir.AxisListType.X,
                                            op=ALU.add)
                if t == 0:
                    nc.vector.tensor_copy(outi[:, bc, 0:4], qsel[:, 0:4])
                else:
                    nc.vector.tensor_copy(outi[:, bc, 4 + t - 1:5 + t - 1],
                                          qsel[:, 0:1])
                if t == delta - 1:
                    return None, None
                qf = qsel[:, 0:1]
                m_ge = wk.tile([128, 100], F32, tag="mge", name="mge", bufs=1)
                nc.vector.tensor_scalar(m_ge[:], io100[:], qf, None,
                                        op0=ALU.is_le)
                qm = wk.tile([128, 1], F32, tag="qm", name="qm")
                nc.vector.tensor_scalar(qm[:], qf, -100.0, None, op0=ALU.add)
                m_lt = wk.tile([128, 100], F32, tag="mlt", name="mlt", bufs=1)
                nc.vector.tensor_scalar(m_lt[:], io100[:], qm[:], None,
                                        op0=ALU.is_gt)
                ohw = wk.tile([128, 100], F32, tag="ohw", name="ohw", bufs=2)
                nc.vector.tensor_mul(ohw[:], m_ge[:], m_lt[:])
                tm = wk.tile([128, 100], F32, tag="tm", name="tm", bufs=1)
                nc.vector.tensor_mul(tm[:], ohw[:], io_f[:])
                fwf = wk.tile([128, 1], F32, tag="fwf", name="fwf")
                nc.vector.tensor_reduce(fwf[:], tm[:], axis=mybir.AxisListType.X,
                                        op=ALU.add)
                flf = wk.tile([128, 1], F32, tag="flf", name="flf")
                nc.vector.tensor_scalar(flf[:], fwf[:], -100.0, qf,
                                        op0=ALU.mult, op1=ALU.add)
                ohl = wk.tile([128, 100], F32, tag="ohl", name="ohl", bufs=2)
                nc.vector.tensor_scalar(ohl[:], io_f[:], flf[:], None,
                                        op0=ALU.is_equal)
                return ohw, ohl

            def trans_oh(bc, ohw, ohl):
                """transpose one-hots into [100, BS] bf16 table operands."""
                bs = bsl(bc)
                pw = p3.tile([128, 512], F32, tag="fc3p", name="ptw")
                nc.tensor.transpose(pw[0:100, 0:128], ohw[:], ident[:])
                nc.vector.tensor_copy(ohwT[:, bs], pw[0:100, 0:128])
                pl = p3.tile([128, 512], F32, tag="fc3p", name="ptl")
                nc.tensor.transpose(pl[0:100, 0:128], ohl[:], ident[:])
                nc.vector.tensor_copy(ohlT[:, bs], pl[0:100, 0:128])

            # ================= main loop (software-pipelined) =============
            # LSTM of step 0 (x-path prologue)
            a0 = gate_acts(0, gates_layer1(0, 0))
            cell_update(0, a0, c1_t, h1_t, h1h, h1l)
            a1 = gate_acts(1, gates_layer1(1, 0))
            cell_update(1, a1, c1_t, h1_t, h1h, h1l)
            a0 = gate_acts(0, gates_layer2(0))
            cell_update(0, a0, c2_t, h2_t, h2h, h2l)
            a1 = gate_acts(1, gates_layer2(1))
            cell_update(1, a1, c2_t, h2_t, h2h, h2l)

            for t in range(delta):
                last = (t == delta - 1)
                cv0, ci0 = fc3_scan(0)
                m0 = merge_onehot(0, t, cv0, ci0)
                cv1, ci1 = fc3_scan(1)
                if not last:
                    # chunk A of step t+1's LSTM, interleaved with chunk B's
                    # merge so PE keeps streaming while DVE merges
                    trans_oh(0, m0[0], m0[1])
                    g1a = gates_layer1(0, t + 1)
                    aa = gate_acts(0, g1a)
                m1 = merge_onehot(1, t, cv1, ci1)
                if not last:
                    trans_oh(1, m1[0], m1[1])
                    g1b = gates_layer1(1, t + 1)
                    ab = gate_acts(1, g1b)
                    cell_update(0, aa, c1_t, h1_t, h1h, h1l)
                    cell_update(1, ab, c1_t, h1_t, h1h, h1l)
                    a2a = gate_acts(0, gates_layer2(0))
                    cell_update(0, a2a, c2_t, h2_t, h2h, h2l)
                    a2b = gate_acts(1, gates_layer2(1))
                    cell_update(1, a2b, c2_t, h2_t, h2h, h2l)

            for bc in range(2):
                nc.sync.dma_start(idx_out[bc], outi[:, bc, :])
    nc.finalize()
    return nc


def _prep_shared(inputs):
    f32, f64 = np.float32, np.float64
    bf = ml_dtypes.bfloat16

    def split(a):
        ah = a.astype(bf)
        al = (a.astype(f32) - ah.astype(f32)).astype(bf)
        return ah, al

    def triple(a):  # fp32 vector -> [3, n] bf16 triple summing to a
        a = a.astype(f32)
        t0 = a.astype(bf)
        r1 = (a - t0.astype(f32)).astype(f32)
        t1 = r1.astype(bf)
        t2 = (r1 - t1.astype(f32)).astype(bf)
        return np.ascontiguousarray(np.stack([t0, t1, t2]))

    def fmT(w):  # [out, in] -> lhsT chunks [2, 128, out]
        wt = np.ascontiguousarray(w.T.astype(f32))
        return wt.reshape(2, 128, wt.shape[1])

    W = {k: np.asarray(v) for k, v in inputs.items()}
    fc1, fc2, fc3 = (W['fc1_W'].astype(f64), W['fc2_W'].astype(f64),
                     W['fc3_W'].astype(f64))
    W3f = (fc3 @ fc2 @ fc1).astype(f32)                       # [Q, 256]
    b3f = (W['fc3_b'].astype(f64) + fc3 @ W['fc2_b'].astype(f64)
           + (fc3 @ fc2) @ W['fc1_b'].astype(f64)).astype(f32)
    Aw = (W['lstm1_Wih'][:, :128].astype(f64)
          @ W['fcqw_W'].astype(f64)).astype(f32)              # [1024, 100]
    Al = (W['lstm1_Wih'][:, 128:].astype(f64)
          @ W['fcql_W'].astype(f64)).astype(f32)
    embb = np.concatenate([W['fcqw_b'], W['fcql_b']]).astype(f64)
    b1f = (W['lstm1_b'].astype(f64)
           + W['lstm1_Wih'].astype(f64) @ embb).astype(f32)

    shared = {}
    for name, w in (("w1ih", W['lstm1_Wih']), ("w1hh", W['lstm1_Whh']),
                    ("w2ih", W['lstm2_Wih']), ("w2hh", W['lstm2_Whh'])):
        h_, l_ = split(fmT(w))
        shared[name + "Th"] = h_
        shared[name + "Tl"] = l_
    shared["w3Th"], shared["w3Tl"] = split(fmT(W3f))
    shared["awTh"], shared["awTl"] = split(np.ascontiguousarray(Aw.T))
    shared["alTh"], shared["alTl"] = split(np.ascontiguousarray(Al.T))
    shared["b1t"] = triple(W['lstm1_b'])
    shared["b1ft"] = triple(b1f)
    shared["b2t"] = triple(W['lstm2_b'])
    shared["b3t"] = triple(b3f)
    return shared


def _per_core(inputs, c):
    f32 = np.float32
    bf = ml_dtypes.bfloat16
    sl = slice(c * BS, (c + 1) * BS)

    def fmT(a):  # [BS, 256] -> [2, 128, BS]
        return np.ascontiguousarray(a.T.astype(f32)).reshape(2, 128, BS)

    def split(a):
        ah = a.astype(bf)
        al = (a - ah.astype(f32)).astype(bf)
        return ah, al

    x = fmT(np.asarray(inputs["x"])[sl, 0, :])
    h1 = fmT(np.asarray(inputs["h1"])[0, sl])
    h2 = fmT(np.asarray(inputs["h2"])[0, sl])
    xh, xl = split(x)
    h1h, h1l = split(h1)
    h2h, h2l = split(h2)
    return {
        "xh": xh, "xl": xl,
        "c1_fm": fmT(np.asarray(inputs["c1"])[0, sl]),
        "c2_fm": fmT(np.asarray(inputs["c2"])[0, sl]),
        "h1h": h1h, "h1l": h1l, "h2h": h2h, "h2l": h2l,
    }


def kernel(**inputs):
    key = "nc"
    if key not in _CACHE:
        _CACHE[key] = _build_nc()
    nc = _CACHE[key]

    shared = _prep_shared(inputs)
    in_maps = []
    for c in range(NCORES):
        m = dict(shared)
        m.update(_per_core(inputs, c))
        in_maps.append(m)

    from concourse.bass_utils import run_bass_kernel_spmd
    res = run_bass_kernel_spmd(nc, in_maps, list(range(NCORES)))
    return assemble(res.results)


def assemble(results):
    traj = np.zeros((B, DELTA, K4, 2), np.float32)
    for c, r in enumerate(results):
        idx = r["idx_out"].reshape(2, 128, 20).astype(np.int64)
        for bc in range(2):
            rows = slice(c * BS + bc * 128, c * BS + (bc + 1) * 128)
            top4 = idx[bc, :, 0:4]
            traj[rows, 0, :, 0] = (top4 % QL).astype(np.float32)
            traj[rows, 0, :, 1] = (top4 // QL).astype(np.float32)
            greedy = idx[bc, :, 4:4 + DELTA - 1]
            traj[rows, 1:, 0, 0] = (greedy % QL).astype(np.float32)
            traj[rows, 1:, 0, 1] = (greedy // QL).astype(np.float32)
    return traj


# revision 47
# speedup vs baseline: 1.2430x; 1.2430x over previous
"""Trainium2 Bass kernel for nn_Decoder (2-layer LSTM + 3 FC + top-k decode).

Strategy: pure data parallelism over batch (2048 -> 8 cores x 256).
Feature-major activations [feat, batch]. All matmuls are 3-term bf16
splits (hi/lo), empirically exact for every argmax decision. fc1/fc2/fc3
fold on the host into one 256->10000 matmul (fp64 compose). For steps
>= 1 the LSTM1 input matmul becomes one-hot table matmuls
(tables = W1ih @ fcq{w,l}_W, host fp64). The fc3 bias is seeded into
PSUM by a K=3 ones-matmul of a bf16 bias triple, and the top-k scan
(max8 + find_index8) reads PSUM directly - logits never touch SBUF.
The decode pipeline is split into two 128-row chunks and
software-pipelined: the next step's LSTM work is emitted between this
step's per-chunk merge/one-hot phases so PE never drains.
"""
import numpy as np
import ml_dtypes

B, D, H = 2048, 256, 256
K4, QW, QL, DELTA = 4, 100, 100, 16
Q = QW * QL
NCORES = 8
BS = B // NCORES          # 256 rows per core
TW = 500                  # fc3 tile width
NT = Q // TW              # 20 tiles per chunk
NG = 10                   # psum scan groups of 2 tiles (1000 logits)
G4 = 4 * H                # 1024 gates

_CACHE = {}


def _build_nc(delta=DELTA, dbg=False):
    import concourse.mybir as mybir
    import concourse.tile as tile
    import concourse.bacc as bacc
    from concourse.masks import make_identity

    F32 = mybir.dt.float32
    BF16 = mybir.dt.bfloat16
    U32 = mybir.dt.uint32
    AF = mybir.ActivationFunctionType
    ALU = mybir.AluOpType

    nc = bacc.Bacc(None, target_bir_lowering=False, debug=False)

    def din(name, shape, dt=F32):
        return nc.dram_tensor(name, shape, dt, kind="ExternalInput")

    # per-core inputs
    xh_in = din("xh", [2, 128, BS], BF16)
    xl_in = din("xl", [2, 128, BS], BF16)
    c1_in = din("c1_fm", [2, 128, BS])
    c2_in = din("c2_fm", [2, 128, BS])
    h1h_in = din("h1h", [2, 128, BS], BF16)
    h1l_in = din("h1l", [2, 128, BS], BF16)
    h2h_in = din("h2h", [2, 128, BS], BF16)
    h2l_in = din("h2l", [2, 128, BS], BF16)
    # shared weights (bf16 hi/lo pairs, lhsT layout)
    w1ihh_in = din("w1ihTh", [2, 128, G4], BF16)
    w1ihl_in = din("w1ihTl", [2, 128, G4], BF16)
    w1hhh_in = din("w1hhTh", [2, 128, G4], BF16)
    w1hhl_in = din("w1hhTl", [2, 128, G4], BF16)
    w2ihh_in = din("w2ihTh", [2, 128, G4], BF16)
    w2ihl_in = din("w2ihTl", [2, 128, G4], BF16)
    w2hhh_in = din("w2hhTh", [2, 128, G4], BF16)
    w2hhl_in = din("w2hhTl", [2, 128, G4], BF16)
    w3h_in = din("w3Th", [2, 128, Q], BF16)
    w3l_in = din("w3Tl", [2, 128, Q], BF16)
    awh_in = din("awTh", [100, G4], BF16)
    awl_in = din("awTl", [100, G4], BF16)
    alh_in = din("alTh", [100, G4], BF16)
    all_in = din("alTl", [100, G4], BF16)
    b1r_in = din("b1r", [128, 8])
    b1rf_in = din("b1rf", [128, 8])
    b2r_in = din("b2r", [128, 8])
    b3t_in = din("b3t", [3, Q], mybir.dt.bfloat16)
    iob80_in = din("iob80", [128, 80])

    idx_out = nc.dram_tensor("idx_out", [2, 128, 20], U32, kind="ExternalOutput")
    if dbg:
        dbg_lq = nc.dram_tensor("dbg_lq", [128, 1024], F32, kind="ExternalOutput")
        dbg_cv = nc.dram_tensor("dbg_cv", [128, 80], F32, kind="ExternalOutput")
        dbg_ci = nc.dram_tensor("dbg_ci", [128, 80], F32, kind="ExternalOutput")
        dbg_h2 = nc.dram_tensor("dbg_h2", [128, 2, BS], F32, kind="ExternalOutput")
        dbg_h1 = nc.dram_tensor("dbg_h1", [128, 2, BS], F32, kind="ExternalOutput")
        dbg_c1 = nc.dram_tensor("dbg_c1", [128, 2, BS], F32, kind="ExternalOutput")
        dbg_g1 = nc.dram_tensor("dbg_g1", [128, 8, 128], F32, kind="ExternalOutput")

    with tile.TileContext(nc) as tc:
        with (
            tc.tile_pool(name="wp", bufs=1) as wp,
            tc.tile_pool(name="st", bufs=1) as st,
            tc.tile_pool(name="wk", bufs=2) as wk,
            tc.tile_pool(name="p3", bufs=2, space="PSUM") as p3,
            tc.tile_pool(name="pg", bufs=2, space="PSUM") as pg,
        ):
            # ---- weight / const loads (ordered by first use) ----
            def wload(src, shape, tag, dt=F32):
                t = wp.tile(shape, dt, tag=tag, name=tag)
                if len(shape) == 3 and shape[1] == 2:
                    nc.sync.dma_start(t[:], src[:].rearrange("c p f -> p c f"))
                else:
                    nc.sync.dma_start(t[:], src[:])
                return t

            w1ihh = wload(w1ihh_in, [128, 2, G4], "w1ihh", BF16)
            w1ihl = wload(w1ihl_in, [128, 2, G4], "w1ihl", BF16)
            w1hhh = wload(w1hhh_in, [128, 2, G4], "w1hhh", BF16)
            w1hhl = wload(w1hhl_in, [128, 2, G4], "w1hhl", BF16)
            b1r = wload(b1r_in, [128, 8], "b1r")
            b2r = wload(b2r_in, [128, 8], "b2r")
            w2ihh = wload(w2ihh_in, [128, 2, G4], "w2ihh", BF16)
            w2ihl = wload(w2ihl_in, [128, 2, G4], "w2ihl", BF16)
            w2hhh = wload(w2hhh_in, [128, 2, G4], "w2hhh", BF16)
            w2hhl = wload(w2hhl_in, [128, 2, G4], "w2hhl", BF16)
            w3h = wload(w3h_in, [128, 2, Q], "w3h", BF16)
            w3l = wload(w3l_in, [128, 2, Q], "w3l", BF16)
            b3t = wload(b3t_in, [3, Q], "b3t", BF16)
            iob80 = wload(iob80_in, [128, 80], "iob80")
            awh = wload(awh_in, [100, G4], "awh", BF16)
            awl = wload(awl_in, [100, G4], "awl", BF16)
            alh = wload(alh_in, [100, G4], "alh", BF16)
            all_ = wload(all_in, [100, G4], "all", BF16)
            b1rf = wload(b1rf_in, [128, 8], "b1rf")

            one3 = wp.tile([3, 128], BF16)
            nc.vector.memset(one3[:], 1.0)
            ident = wp.tile([128, 128], F32)
            make_identity(nc, ident[:])
            io_f = wp.tile([128, 100], F32)
            nc.gpsimd.iota(io_f[:], pattern=[[1, 100]], base=0,
                           channel_multiplier=0,
                           allow_small_or_imprecise_dtypes=True)
            io100 = wp.tile([128, 100], F32)
            nc.gpsimd.iota(io100[:], pattern=[[100, 100]], base=0,
                           channel_multiplier=0,
                           allow_small_or_imprecise_dtypes=True)
            io80 = wp.tile([128, 80], F32)
            nc.gpsimd.iota(io80[:], pattern=[[1, 80]], base=0,
                           channel_multiplier=0,
                           allow_small_or_imprecise_dtypes=True)

            # ---- persistent state ----
            def sload(src, tag, dt=F32):
                t = st.tile([128, 2, BS], dt, tag=tag, name=tag)
                nc.sync.dma_start(t[:], src[:].rearrange("c p b -> p c b"))
                return t

            xh = sload(xh_in, "xh", BF16)
            xl = sload(xl_in, "xl", BF16)
            c1_t = sload(c1_in, "c1")
            c2_t = sload(c2_in, "c2")
            h1h = sload(h1h_in, "h1h", BF16)
            h1l = sload(h1l_in, "h1l", BF16)
            h2h = sload(h2h_in, "h2h", BF16)
            h2l = sload(h2l_in, "h2l", BF16)
            h1_t = st.tile([128, 2, BS], F32, tag="h1", name="h1")
            h2_t = st.tile([128, 2, BS], F32, tag="h2", name="h2")
            ohwT = st.tile([100, BS], BF16, tag="ohwT", name="ohwT")
            ohlT = st.tile([100, BS], BF16, tag="ohlT", name="ohlT")
            outi = st.tile([128, 2, 20], U32, tag="outi", name="outi")
            nc.vector.memset(outi[:], 0)

            def bsl(bc):
                return slice(128 * bc, 128 * (bc + 1))

            # ---- per-chunk LSTM matmul phases ----
            def gates_layer1(bc, t):
                """gates1 psum: recurrent + x (t=0) / one-hot table part."""
                gp = pg.tile([128, 8, 128], F32, tag="g1", name="g1")
                bs = bsl(bc)
                for g in range(8):
                    sl = slice(128 * g, 128 * (g + 1))
                    o = gp[:, g, :]
                    for k in range(2):
                        nc.tensor.matmul(o, w1hhh[:, k, sl], h1h[:, k, bs],
                                         start=(k == 0), stop=False)
                        nc.tensor.matmul(o, w1hhh[:, k, sl], h1l[:, k, bs],
                                         start=False, stop=False)
                        nc.tensor.matmul(o, w1hhl[:, k, sl], h1h[:, k, bs],
                                         start=False, stop=False)
                    if t == 0:
                        for k in range(2):
                            nc.tensor.matmul(o, w1ihh[:, k, sl], xh[:, k, bs],
                                             start=False, stop=False)
                            nc.tensor.matmul(o, w1ihh[:, k, sl], xl[:, k, bs],
                                             start=False, stop=False)
                            nc.tensor.matmul(o, w1ihl[:, k, sl], xh[:, k, bs],
                                             start=False, stop=(k == 1))
                    else:
                        nc.tensor.matmul(o, awh[:, sl], ohwT[:, bs],
                                         start=False, stop=False)
                        nc.tensor.matmul(o, awl[:, sl], ohwT[:, bs],
                                         start=False, stop=False)
                        nc.tensor.matmul(o, alh[:, sl], ohlT[:, bs],
                                         start=False, stop=False)
                        nc.tensor.matmul(o, all_[:, sl], ohlT[:, bs],
                                         start=False, stop=True)
                return gp

            def gates_layer2(bc):
                gp = pg.tile([128, 8, 128], F32, tag="g1", name="g2")
                bs = bsl(bc)
                for g in range(8):
                    sl = slice(128 * g, 128 * (g + 1))
                    o = gp[:, g, :]
                    for k in range(2):
                        nc.tensor.matmul(o, w2ihh[:, k, sl], h1h[:, k, bs],
                                         start=(k == 0), stop=False)
                        nc.tensor.matmul(o, w2ihh[:, k, sl], h1l[:, k, bs],
                                         start=False, stop=False)
                        nc.tensor.matmul(o, w2ihl[:, k, sl], h1h[:, k, bs],
                                         start=False, stop=False)
                    for k in range(2):
                        nc.tensor.matmul(o, w2hhh[:, k, sl], h2h[:, k, bs],
                                         start=False, stop=False)
                        nc.tensor.matmul(o, w2hhh[:, k, sl], h2l[:, k, bs],
                                         start=False, stop=False)
                        nc.tensor.matmul(o, w2hhl[:, k, sl], h2h[:, k, bs],
                                         start=False, stop=(k == 1))
                return gp

            def gate_acts(bc, gp, br):
                """sigmoid/tanh activations with per-slice gate biases."""
                si = wk.tile([128, 2, 128], F32, tag="si")
                sf = wk.tile([128, 2, 128], F32, tag="sf")
                tg = wk.tile([128, 2, 128], F32, tag="tg")
                so = wk.tile([128, 2, 128], F32, tag="so")
                for ch in range(2):
                    nc.scalar.activation(si[:, ch, :], gp[:, 0 + ch, :],
                                         AF.Sigmoid, bias=br[:, 0 + ch:1 + ch])
                    nc.scalar.activation(sf[:, ch, :], gp[:, 2 + ch, :],
                                         AF.Sigmoid, bias=br[:, 2 + ch:3 + ch])
                    nc.scalar.activation(tg[:, ch, :], gp[:, 4 + ch, :],
                                         AF.Tanh, bias=br[:, 4 + ch:5 + ch])
                    nc.scalar.activation(so[:, ch, :], gp[:, 6 + ch, :],
                                         AF.Sigmoid, bias=br[:, 6 + ch:7 + ch])
                return si, sf, tg, so

            def cell_update(bc, acts, cT, hT, hh, hl):
                si, sf, tg, so = acts
                bs = bsl(bc)
                csl = cT[:, :, bs]
                hsl = hT[:, :, bs]
                t1 = wk.tile([128, 2, 128], F32, tag="t1", bufs=1)
                t2 = wk.tile([128, 2, 128], F32, tag="t2", bufs=1)
                nc.vector.tensor_mul(t1[:], sf[:], csl)
                nc.vector.tensor_mul(t2[:], si[:], tg[:])
                nc.vector.tensor_add(csl, t1[:], t2[:])
                t3 = wk.tile([128, 2, 128], F32, tag="t3", bufs=1)
                nc.scalar.activation(t3[:], csl, AF.Tanh)
                nc.vector.tensor_mul(hsl, so[:], t3[:])
                nc.vector.tensor_copy(hh[:, :, bs], hsl)
                nc.vector.tensor_sub(hl[:, :, bs], hsl, hh[:, :, bs])

            # ---- fc3 + PSUM-direct scan for one chunk ----
            # 20 tiles: 19 x 512 + 1 x 272; scan groups of 2 tiles in one
            # flat 2-bank psum tile so indices stay affine (base 1024*gi)
            FTILES = [(i * 512, 512) for i in range(19)] + [(9728, 272)]

            def fc3_scan(bc):
                bs = bsl(bc)
                cand_v = wk.tile([128, 80], F32, tag="candv", name="candv")
                if not hasattr(fc3_scan, "ran"):
                    fc3_scan.ran = [False]
                cand_iu = wk.tile([128, 80], U32, tag="candiu", name="candiu")
                stats = [(h2h[:, 0, bs], w3h[:, 0, :]),
                         (h2h[:, 1, bs], w3h[:, 1, :]),
                         (h2l[:, 0, bs], w3h[:, 0, :]),
                         (h2l[:, 1, bs], w3h[:, 1, :]),
                         (h2h[:, 0, bs], w3l[:, 0, :]),
                         (h2h[:, 1, bs], w3l[:, 1, :])]
                for gi in range(NG):
                    pt = p3.tile([128, 1024], F32, tag="fc3p", name="fc3p")
                    spanw = 0
                    for ti in range(2):
                        n0, wdt = FTILES[2 * gi + ti]
                        o = pt[:, 512 * ti:512 * ti + wdt]
                        # seed PSUM with the fc3 bias triple, then accumulate
                        nc.tensor.matmul(o, one3[:], b3t[:, n0:n0 + wdt],
                                         start=True, stop=False)
                        for j, (stat, w) in enumerate(stats):
                            nc.tensor.matmul(o, stat, w[:, n0:n0 + wdt],
                                             start=False, stop=(j == 5))
                        spanw = 512 * ti + wdt
                    span = pt[:, 0:spanw]
                    if dbg and bc == 0 and gi == 0 and not fc3_scan.ran[0]:
                        lqg = wk.tile([128, 1024], F32, tag="lqg", name="lqg")
                        nc.vector.tensor_copy(lqg[:, 0:spanw], span)
                        nc.sync.dma_start(dbg_lq[:], lqg[:])
                    nc.vector.max(cand_v[:, 8 * gi:8 * gi + 8], span)
                    nc.vector.max_index(cand_iu[:, 8 * gi:8 * gi + 8],
                                        cand_v[:, 8 * gi:8 * gi + 8], span)
                cif = wk.tile([128, 80], F32, tag="cif", name="cif")
                nc.vector.tensor_copy(cif[:], cand_iu[:])
                cand_i = wk.tile([128, 80], F32, tag="candi", name="candi")
                nc.vector.tensor_add(cand_i[:], cif[:], iob80[:])
                if dbg and bc == 0 and not fc3_scan.ran[0]:
                    nc.sync.dma_start(dbg_cv[:], cand_v[:])
                    nc.sync.dma_start(dbg_ci[:], cand_i[:])
                    nc.sync.dma_start(dbg_h2[:], h2_t[:])
                    fc3_scan.ran[0] = True
                return cand_v, cand_i

            def merge_onehot(bc, t, cand_v, cand_i):
                """top-k merge, trajectory index write, one-hot build."""
                vm8 = wk.tile([128, 8], F32, tag="vm8", name="vm8")
                pm8 = wk.tile([128, 8], U32, tag="pm8", name="pm8")
                nc.vector.max(vm8[:], cand_v[:])
                nc.vector.max_index(pm8[:], vm8[:], cand_v[:])
                pmf = wk.tile([128, 8], F32, tag="pmf", name="pmf")
                nc.vector.tensor_copy(pmf[:], pm8[:])
                nk = 4 if t == 0 else 1
                qsel = wk.tile([128, 4], F32, tag="qsel", name="qsel")
                for kk in range(nk):
                    ohp = wk.tile([128, 80], F32, tag="ohp", name="ohp")
                    nc.vector.tensor_scalar(ohp[:], io80[:], pmf[:, kk:kk + 1],
                                            None, op0=ALU.is_equal)
                    tmq = wk.tile([128, 80], F32, tag="tmq", name="tmq")
                    nc.vector.tensor_mul(tmq[:], ohp[:], cand_i[:])
                    nc.vector.tensor_reduce(qsel[:, kk:kk + 1], tmq[:],
                                            axis=mybir.AxisListType.X,
                                            op=ALU.add)
                if t == 0:
                    nc.vector.tensor_copy(outi[:, bc, 0:4], qsel[:, 0:4])
                else:
                    nc.vector.tensor_copy(outi[:, bc, 4 + t - 1:5 + t - 1],
                                          qsel[:, 0:1])
                if t == delta - 1:
                    return None, None
                qf = qsel[:, 0:1]
                m_ge = wk.tile([128, 100], F32, tag="mge", name="mge", bufs=1)
                nc.vector.tensor_scalar(m_ge[:], io100[:], qf, None,
                                        op0=ALU.is_le)
                qm = wk.tile([128, 1], F32, tag="qm", name="qm")
                nc.vector.tensor_scalar(qm[:], qf, -100.0, None, op0=ALU.add)
                m_lt = wk.tile([128, 100], F32, tag="mlt", name="mlt", bufs=1)
                nc.vector.tensor_scalar(m_lt[:], io100[:], qm[:], None,
                                        op0=ALU.is_gt)
                ohw = wk.tile([128, 100], F32, tag="ohw", name="ohw", bufs=2)
                nc.vector.tensor_mul(ohw[:], m_ge[:], m_lt[:])
                tm = wk.tile([128, 100], F32, tag="tm", name="tm", bufs=1)
                nc.vector.tensor_mul(tm[:], ohw[:], io_f[:])
                fwf = wk.tile([128, 1], F32, tag="fwf", name="fwf")
                nc.vector.tensor_reduce(fwf[:], tm[:], axis=mybir.AxisListType.X,
                                        op=ALU.add)
                flf = wk.tile([128, 1], F32, tag="flf", name="flf")
                nc.vector.tensor_scalar(flf[:], fwf[:], -100.0, qf,
                                        op0=ALU.mult, op1=ALU.add)
                ohl = wk.tile([128, 100], F32, tag="ohl", name="ohl", bufs=2)
                nc.vector.tensor_scalar(ohl[:], io_f[:], flf[:], None,
                                        op0=ALU.is_equal)
                return ohw, ohl

            def trans_oh(bc, ohw, ohl):
                """transpose one-hots into [100, BS] bf16 table operands."""
                bs = bsl(bc)
                pw = p3.tile([128, 1024], F32, tag="fc3p", name="ptw")
                nc.tensor.transpose(pw[0:100, 0:128], ohw[:], ident[:])
                nc.vector.tensor_copy(ohwT[:, bs], pw[0:100, 0:128])
                nc.tensor.transpose(pw[0:100, 512:640], ohl[:], ident[:])
                nc.vector.tensor_copy(ohlT[:, bs], pw[0:100, 512:640])

            # ================= main loop (software-pipelined) =============
            # LSTM of step 0 (x-path prologue)
            for bc in range(2):
                gp = gates_layer1(bc, 0)
                if dbg and bc == 0:
                    g1c = wk.tile([128, 8, 128], F32, tag="g1c", name="g1c")
                    nc.vector.tensor_copy(g1c[:], gp[:])
                    nc.sync.dma_start(dbg_g1[:], g1c[:])
                a = gate_acts(bc, gp, b1r)
                cell_update(bc, a, c1_t, h1_t, h1h, h1l)
                if dbg and bc == 0:
                    nc.sync.dma_start(dbg_h1[:], h1_t[:])
                    nc.sync.dma_start(dbg_c1[:], c1_t[:])
            for bc in range(2):
                a = gate_acts(bc, gates_layer2(bc), b2r)
                cell_update(bc, a, c2_t, h2_t, h2h, h2l)

            for t in range(delta):
                last = (t == delta - 1)
                cv0, ci0 = fc3_scan(0)
                m0 = merge_onehot(0, t, cv0, ci0)
                cv1, ci1 = fc3_scan(1)
                if not last:
                    trans_oh(0, m0[0], m0[1])
                    gp1a = gates_layer1(0, t + 1)
                    aa = gate_acts(0, gp1a, b1rf)
                m1 = merge_onehot(1, t, cv1, ci1)
                if not last:
                    trans_oh(1, m1[0], m1[1])
                    gp1b = gates_layer1(1, t + 1)
                    ab = gate_acts(1, gp1b, b1rf)
                    cell_update(0, aa, c1_t, h1_t, h1h, h1l)
                    cell_update(1, ab, c1_t, h1_t, h1h, h1l)
                    a2a = gate_acts(0, gates_layer2(0), b2r)
                    cell_update(0, a2a, c2_t, h2_t, h2h, h2l)
                    a2b = gate_acts(1, gates_layer2(1), b2r)
                    cell_update(1, a2b, c2_t, h2_t, h2h, h2l)

            for bc in range(2):
                nc.sync.dma_start(idx_out[bc], outi[:, bc, :])
    nc.finalize()
    return nc


def _prep_shared(inputs):
    f32, f64 = np.float32, np.float64
    bf = ml_dtypes.bfloat16

    def split(a):
        ah = a.astype(bf)
        al = (a.astype(f32) - ah.astype(f32)).astype(bf)
        return ah, al

    def fmT(w):  # [out, in] -> lhsT chunks [2, 128, out]
        wt = np.ascontiguousarray(w.T.astype(f32))
        return wt.reshape(2, 128, wt.shape[1])

    W = {k: np.asarray(v) for k, v in inputs.items()}
    fc1, fc2, fc3 = (W['fc1_W'].astype(f64), W['fc2_W'].astype(f64),
                     W['fc3_W'].astype(f64))
    W3f = (fc3 @ fc2 @ fc1).astype(f32)                       # [Q, 256]
    b3f = (W['fc3_b'].astype(f64) + fc3 @ W['fc2_b'].astype(f64)
           + (fc3 @ fc2) @ W['fc1_b'].astype(f64)).astype(f32)
    Aw = (W['lstm1_Wih'][:, :128].astype(f64)
          @ W['fcqw_W'].astype(f64)).astype(f32)              # [1024, 100]
    Al = (W['lstm1_Wih'][:, 128:].astype(f64)
          @ W['fcql_W'].astype(f64)).astype(f32)
    embb = np.concatenate([W['fcqw_b'], W['fcql_b']]).astype(f64)
    b1f = (W['lstm1_b'].astype(f64)
           + W['lstm1_Wih'].astype(f64) @ embb).astype(f32)

    shared = {}
    for name, w in (("w1ih", W['lstm1_Wih']), ("w1hh", W['lstm1_Whh']),
                    ("w2ih", W['lstm2_Wih']), ("w2hh", W['lstm2_Whh'])):
        h_, l_ = split(fmT(w))
        shared[name + "Th"] = h_
        shared[name + "Tl"] = l_
    shared["w3Th"], shared["w3Tl"] = split(fmT(W3f))
    shared["awTh"], shared["awTl"] = split(np.ascontiguousarray(Aw.T))
    shared["alTh"], shared["alTl"] = split(np.ascontiguousarray(Al.T))
    shared["b1r"] = W['lstm1_b'].astype(f32).reshape(8, 128).T.copy()
    shared["b1rf"] = b1f.reshape(8, 128).T.copy()
    shared["b2r"] = W['lstm2_b'].astype(f32).reshape(8, 128).T.copy()
    # fc3 bias as 3 bf16 terms (seeded into PSUM via a K=3 ones matmul)
    b3a = b3f.astype(bf)
    r1 = (b3f - b3a.astype(f32)).astype(f32)
    b3b = r1.astype(bf)
    b3c = (r1 - b3b.astype(f32)).astype(bf)
    shared["b3t"] = np.ascontiguousarray(np.stack([b3a, b3b, b3c]))
    # per-candidate-slot global index base: slot 8g+k -> 1024g
    iob = np.repeat(np.arange(NG, dtype=f32) * 1024.0, 8)
    shared["iob80"] = np.ascontiguousarray(np.broadcast_to(iob, (128, 80)))
    return shared


def _per_core(inputs, c):
    f32 = np.float32
    bf = ml_dtypes.bfloat16
    sl = slice(c * BS, (c + 1) * BS)

    def fmT(a):  # [BS, 256] -> [2, 128, BS]
        return np.ascontiguousarray(a.T.astype(f32)).reshape(2, 128, BS)

    def split(a):
        ah = a.astype(bf)
        al = (a - ah.astype(f32)).astype(bf)
        return ah, al

    x = fmT(np.asarray(inputs["x"])[sl, 0, :])
    h1 = fmT(np.asarray(inputs["h1"])[0, sl])
    h2 = fmT(np.asarray(inputs["h2"])[0, sl])
    xh, xl = split(x)
    h1h, h1l = split(h1)
    h2h, h2l = split(h2)
    return {
        "xh": xh, "xl": xl,
        "c1_fm": fmT(np.asarray(inputs["c1"])[0, sl]),
        "c2_fm": fmT(np.asarray(inputs["c2"])[0, sl]),
        "h1h": h1h, "h1l": h1l, "h2h": h2h, "h2l": h2l,
    }


def kernel(**inputs):
    key = "nc"
    if key not in _CACHE:
        _CACHE[key] = _build_nc()
    nc = _CACHE[key]

    shared = _prep_shared(inputs)
    in_maps = []
    for c in range(NCORES):
        m = dict(shared)
        m.update(_per_core(inputs, c))
        in_maps.append(m)

    from concourse.bass_utils import run_bass_kernel_spmd
    res = run_bass_kernel_spmd(nc, in_maps, list(range(NCORES)))
    return assemble(res.results)


def assemble(results):
    traj = np.zeros((B, DELTA, K4, 2), np.float32)
    for c, r in enumerate(results):
        idx = r["idx_out"].reshape(2, 128, 20).astype(np.int64)
        for bc in range(2):
            rows = slice(c * BS + bc * 128, c * BS + (bc + 1) * 128)
            top4 = idx[bc, :, 0:4]
            traj[rows, 0, :, 0] = (top4 % QL).astype(np.float32)
            traj[rows, 0, :, 1] = (top4 // QL).astype(np.float32)
            greedy = idx[bc, :, 4:4 + DELTA - 1]
            traj[rows, 1:, 0, 0] = (greedy % QL).astype(np.float32)
            traj[rows, 1:, 0, 1] = (greedy // QL).astype(np.float32)
    return traj


# revision 49
# speedup vs baseline: 1.2544x; 1.0092x over previous
"""Trainium2 Bass kernel for nn_Decoder (2-layer LSTM + 3 FC + top-k decode).

Strategy: pure data parallelism over batch (2048 -> 8 cores x 256).
Feature-major activations [feat, batch]. All matmuls are 3-term bf16
splits (hi/lo), empirically exact for every argmax decision. fc1/fc2/fc3
fold on the host into one 256->10000 matmul (fp64 compose). For steps
>= 1 the LSTM1 input matmul becomes one-hot table matmuls
(tables = W1ih @ fcq{w,l}_W, host fp64). The fc3 bias is seeded into
PSUM by a K=3 ones-matmul of a bf16 bias triple, and the top-k scan
(max8 + find_index8) reads PSUM directly - logits never touch SBUF.
The decode pipeline is split into two 128-row chunks and
software-pipelined: the next step's LSTM work is emitted between this
step's per-chunk merge/one-hot phases so PE never drains.
"""
import numpy as np
import ml_dtypes

B, D, H = 2048, 256, 256
K4, QW, QL, DELTA = 4, 100, 100, 16
Q = QW * QL
NCORES = 8
BS = B // NCORES          # 256 rows per core
TW = 500                  # fc3 tile width
NT = Q // TW              # 20 tiles per chunk
NG = 10                   # psum scan groups of 2 tiles (1000 logits)
G4 = 4 * H                # 1024 gates

_CACHE = {}


def _build_nc(delta=DELTA, dbg=False):
    import concourse.mybir as mybir
    import concourse.tile as tile
    import concourse.bacc as bacc
    from concourse.masks import make_identity

    F32 = mybir.dt.float32
    BF16 = mybir.dt.bfloat16
    U32 = mybir.dt.uint32
    AF = mybir.ActivationFunctionType
    ALU = mybir.AluOpType

    nc = bacc.Bacc(None, target_bir_lowering=False, debug=False)

    def din(name, shape, dt=F32):
        return nc.dram_tensor(name, shape, dt, kind="ExternalInput")

    # per-core inputs
    xh_in = din("xh", [2, 128, BS], BF16)
    xl_in = din("xl", [2, 128, BS], BF16)
    c1_in = din("c1_fm", [2, 128, BS])
    c2_in = din("c2_fm", [2, 128, BS])
    h1h_in = din("h1h", [2, 128, BS], BF16)
    h1l_in = din("h1l", [2, 128, BS], BF16)
    h2h_in = din("h2h", [2, 128, BS], BF16)
    h2l_in = din("h2l", [2, 128, BS], BF16)
    # shared weights (bf16 hi/lo pairs, lhsT layout)
    w1ihh_in = din("w1ihTh", [2, 128, G4], BF16)
    w1ihl_in = din("w1ihTl", [2, 128, G4], BF16)
    w1hhh_in = din("w1hhTh", [2, 128, G4], BF16)
    w1hhl_in = din("w1hhTl", [2, 128, G4], BF16)
    w2ihh_in = din("w2ihTh", [2, 128, G4], BF16)
    w2ihl_in = din("w2ihTl", [2, 128, G4], BF16)
    w2hhh_in = din("w2hhTh", [2, 128, G4], BF16)
    w2hhl_in = din("w2hhTl", [2, 128, G4], BF16)
    w3h_in = din("w3Th", [2, 128, Q], BF16)
    w3l_in = din("w3Tl", [2, 128, Q], BF16)
    awh_in = din("awTh", [100, G4], BF16)
    awl_in = din("awTl", [100, G4], BF16)
    alh_in = din("alTh", [100, G4], BF16)
    all_in = din("alTl", [100, G4], BF16)
    b1r_in = din("b1r", [128, 8])
    b1rf_in = din("b1rf", [128, 8])
    b2r_in = din("b2r", [128, 8])
    b3t_in = din("b3t", [3, Q], mybir.dt.bfloat16)
    iob80_in = din("iob80", [128, 80])

    idx_out = nc.dram_tensor("idx_out", [2, 128, 20], U32, kind="ExternalOutput")
    if dbg:
        dbg_lq = nc.dram_tensor("dbg_lq", [128, 1024], F32, kind="ExternalOutput")
        dbg_cv = nc.dram_tensor("dbg_cv", [128, 80], F32, kind="ExternalOutput")
        dbg_ci = nc.dram_tensor("dbg_ci", [128, 80], F32, kind="ExternalOutput")
        dbg_h2 = nc.dram_tensor("dbg_h2", [128, 2, BS], F32, kind="ExternalOutput")
        dbg_h1 = nc.dram_tensor("dbg_h1", [128, 2, BS], F32, kind="ExternalOutput")
        dbg_c1 = nc.dram_tensor("dbg_c1", [128, 2, BS], F32, kind="ExternalOutput")
        dbg_g1 = nc.dram_tensor("dbg_g1", [128, 8, 128], F32, kind="ExternalOutput")

    with tile.TileContext(nc) as tc:
        with (
            tc.tile_pool(name="wp", bufs=1) as wp,
            tc.tile_pool(name="st", bufs=1) as st,
            tc.tile_pool(name="wk", bufs=2) as wk,
            tc.tile_pool(name="p3", bufs=2, space="PSUM") as p3,
            tc.tile_pool(name="pg", bufs=2, space="PSUM") as pg,
        ):
            # ---- weight / const loads (ordered by first use) ----
            def wload(src, shape, tag, dt=F32):
                t = wp.tile(shape, dt, tag=tag, name=tag)
                if len(shape) == 3 and shape[1] == 2:
                    nc.sync.dma_start(t[:], src[:].rearrange("c p f -> p c f"))
                else:
                    nc.sync.dma_start(t[:], src[:])
                return t

            w1ihh = wload(w1ihh_in, [128, 2, G4], "w1ihh", BF16)
            w1ihl = wload(w1ihl_in, [128, 2, G4], "w1ihl", BF16)
            w1hhh = wload(w1hhh_in, [128, 2, G4], "w1hhh", BF16)
            w1hhl = wload(w1hhl_in, [128, 2, G4], "w1hhl", BF16)
            b1r = wload(b1r_in, [128, 8], "b1r")
            b2r = wload(b2r_in, [128, 8], "b2r")

            one3 = wp.tile([3, 128], BF16)
            nc.vector.memset(one3[:], 1.0)
            ident = wp.tile([128, 128], F32)
            make_identity(nc, ident[:])
            io_f = wp.tile([128, 100], F32)
            nc.gpsimd.iota(io_f[:], pattern=[[1, 100]], base=0,
                           channel_multiplier=0,
                           allow_small_or_imprecise_dtypes=True)
            io100 = wp.tile([128, 100], F32)
            nc.gpsimd.iota(io100[:], pattern=[[100, 100]], base=0,
                           channel_multiplier=0,
                           allow_small_or_imprecise_dtypes=True)
            io80 = wp.tile([128, 80], F32)
            nc.gpsimd.iota(io80[:], pattern=[[1, 80]], base=0,
                           channel_multiplier=0,
                           allow_small_or_imprecise_dtypes=True)

            # ---- persistent state ----
            def sload(src, tag, dt=F32):
                t = st.tile([128, 2, BS], dt, tag=tag, name=tag)
                nc.sync.dma_start(t[:], src[:].rearrange("c p b -> p c b"))
                return t

            xh = sload(xh_in, "xh", BF16)
            xl = sload(xl_in, "xl", BF16)
            c1_t = sload(c1_in, "c1")
            c2_t = sload(c2_in, "c2")
            h1h = sload(h1h_in, "h1h", BF16)
            h1l = sload(h1l_in, "h1l", BF16)
            h2h = sload(h2h_in, "h2h", BF16)
            h2l = sload(h2l_in, "h2l", BF16)
            # bulk weights after the step-0 dependencies
            w2ihh = wload(w2ihh_in, [128, 2, G4], "w2ihh", BF16)
            w2ihl = wload(w2ihl_in, [128, 2, G4], "w2ihl", BF16)
            w2hhh = wload(w2hhh_in, [128, 2, G4], "w2hhh", BF16)
            w2hhl = wload(w2hhl_in, [128, 2, G4], "w2hhl", BF16)
            b3t = wload(b3t_in, [3, Q], "b3t", BF16)
            w3h = wload(w3h_in, [128, 2, Q], "w3h", BF16)
            w3l = wload(w3l_in, [128, 2, Q], "w3l", BF16)
            iob80 = wload(iob80_in, [128, 80], "iob80")
            awh = wload(awh_in, [100, G4], "awh", BF16)
            awl = wload(awl_in, [100, G4], "awl", BF16)
            alh = wload(alh_in, [100, G4], "alh", BF16)
            all_ = wload(all_in, [100, G4], "all", BF16)
            b1rf = wload(b1rf_in, [128, 8], "b1rf")
            h1_t = st.tile([128, 2, BS], F32, tag="h1", name="h1")
            h2_t = st.tile([128, 2, BS], F32, tag="h2", name="h2")
            ohwT = st.tile([100, BS], BF16, tag="ohwT", name="ohwT")
            ohlT = st.tile([100, BS], BF16, tag="ohlT", name="ohlT")
            outi = st.tile([128, 2, 20], U32, tag="outi", name="outi")
            nc.vector.memset(outi[:], 0)

            def bsl(bc):
                return slice(128 * bc, 128 * (bc + 1))

            # ---- per-chunk LSTM matmul phases ----
            def gates_layer1(bc, t):
                """gates1 psum: recurrent + x (t=0) / one-hot table part."""
                gp = pg.tile([128, 8, 128], F32, tag="g1", name="g1")
                bs = bsl(bc)
                for g in range(8):
                    sl = slice(128 * g, 128 * (g + 1))
                    o = gp[:, g, :]
                    for k in range(2):
                        nc.tensor.matmul(o, w1hhh[:, k, sl], h1h[:, k, bs],
                                         start=(k == 0), stop=False)
                        nc.tensor.matmul(o, w1hhh[:, k, sl], h1l[:, k, bs],
                                         start=False, stop=False)
                        nc.tensor.matmul(o, w1hhl[:, k, sl], h1h[:, k, bs],
                                         start=False, stop=False)
                    if t == 0:
                        for k in range(2):
                            nc.tensor.matmul(o, w1ihh[:, k, sl], xh[:, k, bs],
                                             start=False, stop=False)
                            nc.tensor.matmul(o, w1ihh[:, k, sl], xl[:, k, bs],
                                             start=False, stop=False)
                            nc.tensor.matmul(o, w1ihl[:, k, sl], xh[:, k, bs],
                                             start=False, stop=(k == 1))
                    else:
                        nc.tensor.matmul(o, awh[:, sl], ohwT[:, bs],
                                         start=False, stop=False)
                        nc.tensor.matmul(o, awl[:, sl], ohwT[:, bs],
                                         start=False, stop=False)
                        nc.tensor.matmul(o, alh[:, sl], ohlT[:, bs],
                                         start=False, stop=False)
                        nc.tensor.matmul(o, all_[:, sl], ohlT[:, bs],
                                         start=False, stop=True)
                return gp

            def gates_layer2(bc):
                gp = pg.tile([128, 8, 128], F32, tag="g1", name="g2")
                bs = bsl(bc)
                for g in range(8):
                    sl = slice(128 * g, 128 * (g + 1))
                    o = gp[:, g, :]
                    for k in range(2):
                        nc.tensor.matmul(o, w2ihh[:, k, sl], h1h[:, k, bs],
                                         start=(k == 0), stop=False)
                        nc.tensor.matmul(o, w2ihh[:, k, sl], h1l[:, k, bs],
                                         start=False, stop=False)
                        nc.tensor.matmul(o, w2ihl[:, k, sl], h1h[:, k, bs],
                                         start=False, stop=False)
                    for k in range(2):
                        nc.tensor.matmul(o, w2hhh[:, k, sl], h2h[:, k, bs],
                                         start=False, stop=False)
                        nc.tensor.matmul(o, w2hhh[:, k, sl], h2l[:, k, bs],
                                         start=False, stop=False)
                        nc.tensor.matmul(o, w2hhl[:, k, sl], h2h[:, k, bs],
                                         start=False, stop=(k == 1))
                return gp

            def gate_acts(bc, gp, br):
                """sigmoid/tanh activations with per-slice gate biases."""
                si = wk.tile([128, 2, 128], F32, tag="si")
                sf = wk.tile([128, 2, 128], F32, tag="sf")
                tg = wk.tile([128, 2, 128], F32, tag="tg")
                so = wk.tile([128, 2, 128], F32, tag="so")
                for ch in range(2):
                    nc.scalar.activation(si[:, ch, :], gp[:, 0 + ch, :],
                                         AF.Sigmoid, bias=br[:, 0 + ch:1 + ch])
                    nc.scalar.activation(sf[:, ch, :], gp[:, 2 + ch, :],
                                         AF.Sigmoid, bias=br[:, 2 + ch:3 + ch])
                    nc.scalar.activation(tg[:, ch, :], gp[:, 4 + ch, :],
                                         AF.Tanh, bias=br[:, 4 + ch:5 + ch])
                    nc.scalar.activation(so[:, ch, :], gp[:, 6 + ch, :],
                                         AF.Sigmoid, bias=br[:, 6 + ch:7 + ch])
                return si, sf, tg, so

            def cell_update(bc, acts, cT, hT, hh, hl):
                si, sf, tg, so = acts
                bs = bsl(bc)
                csl = cT[:, :, bs]
                hsl = hT[:, :, bs]
                t1 = wk.tile([128, 2, 128], F32, tag="t1", bufs=1)
                t2 = wk.tile([128, 2, 128], F32, tag="t2", bufs=1)
                nc.vector.tensor_mul(t1[:], sf[:], csl)
                nc.vector.tensor_mul(t2[:], si[:], tg[:])
                nc.vector.tensor_add(csl, t1[:], t2[:])
                t3 = wk.tile([128, 2, 128], F32, tag="t3", bufs=1)
                nc.scalar.activation(t3[:], csl, AF.Tanh)
                nc.vector.tensor_mul(hsl, so[:], t3[:])
                nc.vector.tensor_copy(hh[:, :, bs], hsl)
                nc.vector.tensor_sub(hl[:, :, bs], hsl, hh[:, :, bs])

            # ---- fc3 + PSUM-direct scan for one chunk ----
            # 20 tiles: 19 x 512 + 1 x 272; scan groups of 2 tiles in one
            # flat 2-bank psum tile so indices stay affine (base 1024*gi)
            FTILES = [(i * 512, 512) for i in range(19)] + [(9728, 272)]

            def fc3_scan(bc):
                bs = bsl(bc)
                cand_v = wk.tile([128, 80], F32, tag="candv", name="candv")
                if not hasattr(fc3_scan, "ran"):
                    fc3_scan.ran = [False]
                cand_iu = wk.tile([128, 80], U32, tag="candiu", name="candiu")
                stats = [(h2h[:, 0, bs], w3h[:, 0, :]),
                         (h2h[:, 1, bs], w3h[:, 1, :]),
                         (h2l[:, 0, bs], w3h[:, 0, :]),
                         (h2l[:, 1, bs], w3h[:, 1, :]),
                         (h2h[:, 0, bs], w3l[:, 0, :]),
                         (h2h[:, 1, bs], w3l[:, 1, :])]
                for gi in range(NG):
                    pt = p3.tile([128, 1024], F32, tag="fc3p", name="fc3p")
                    spanw = 0
                    for ti in range(2):
                        n0, wdt = FTILES[2 * gi + ti]
                        o = pt[:, 512 * ti:512 * ti + wdt]
                        # seed PSUM with the fc3 bias triple, then accumulate
                        nc.tensor.matmul(o, one3[:], b3t[:, n0:n0 + wdt],
                                         start=True, stop=False)
                        for j, (stat, w) in enumerate(stats):
                            nc.tensor.matmul(o, stat, w[:, n0:n0 + wdt],
                                             start=False, stop=(j == 5))
                        spanw = 512 * ti + wdt
                    span = pt[:, 0:spanw]
                    if dbg and bc == 0 and gi == 0 and not fc3_scan.ran[0]:
                        lqg = wk.tile([128, 1024], F32, tag="lqg", name="lqg")
                        nc.vector.tensor_copy(lqg[:, 0:spanw], span)
                        nc.sync.dma_start(dbg_lq[:], lqg[:])
                    nc.vector.max(cand_v[:, 8 * gi:8 * gi + 8], span)
                    nc.vector.max_index(cand_iu[:, 8 * gi:8 * gi + 8],
                                        cand_v[:, 8 * gi:8 * gi + 8], span)
                cif = wk.tile([128, 80], F32, tag="cif", name="cif")
                nc.vector.tensor_copy(cif[:], cand_iu[:])
                cand_i = wk.tile([128, 80], F32, tag="candi", name="candi")
                nc.vector.tensor_add(cand_i[:], cif[:], iob80[:])
                if dbg and bc == 0 and not fc3_scan.ran[0]:
                    nc.sync.dma_start(dbg_cv[:], cand_v[:])
                    nc.sync.dma_start(dbg_ci[:], cand_i[:])
                    nc.sync.dma_start(dbg_h2[:], h2_t[:])
                    fc3_scan.ran[0] = True
                return cand_v, cand_i

            def merge_onehot(bc, t, cand_v, cand_i):
                """top-k merge, trajectory index write, one-hot build."""
                vm8 = wk.tile([128, 8], F32, tag="vm8", name="vm8")
                pm8 = wk.tile([128, 8], U32, tag="pm8", name="pm8")
                nc.vector.max(vm8[:], cand_v[:])
                nc.vector.max_index(pm8[:], vm8[:], cand_v[:])
                pmf = wk.tile([128, 8], F32, tag="pmf", name="pmf")
                nc.vector.tensor_copy(pmf[:], pm8[:])
                nk = 4 if t == 0 else 1
                qsel = wk.tile([128, 4], F32, tag="qsel", name="qsel")
                for kk in range(nk):
                    ohp = wk.tile([128, 80], F32, tag="ohp", name="ohp")
                    nc.vector.tensor_scalar(ohp[:], io80[:], pmf[:, kk:kk + 1],
                                            None, op0=ALU.is_equal)
                    tmq = wk.tile([128, 80], F32, tag="tmq", name="tmq")
                    nc.vector.tensor_mul(tmq[:], ohp[:], cand_i[:])
                    nc.vector.tensor_reduce(qsel[:, kk:kk + 1], tmq[:],
                                            axis=mybir.AxisListType.X,
                                            op=ALU.add)
                if t == 0:
                    nc.vector.tensor_copy(outi[:, bc, 0:4], qsel[:, 0:4])
                else:
                    nc.vector.tensor_copy(outi[:, bc, 4 + t - 1:5 + t - 1],
                                          qsel[:, 0:1])
                if t == delta - 1:
                    return None, None
                qf = qsel[:, 0:1]
                m_ge = wk.tile([128, 100], F32, tag="mge", name="mge", bufs=1)
                nc.vector.tensor_scalar(m_ge[:], io100[:], qf, None,
                                        op0=ALU.is_le)
                qm = wk.tile([128, 1], F32, tag="qm", name="qm")
                nc.vector.tensor_scalar(qm[:], qf, -100.0, None, op0=ALU.add)
                m_lt = wk.tile([128, 100], F32, tag="mlt", name="mlt", bufs=1)
                nc.vector.tensor_scalar(m_lt[:], io100[:], qm[:], None,
                                        op0=ALU.is_gt)
                ohw = wk.tile([128, 100], F32, tag="ohw", name="ohw", bufs=2)
                nc.vector.tensor_mul(ohw[:], m_ge[:], m_lt[:])
                tm = wk.tile([128, 100], F32, tag="tm", name="tm", bufs=1)
                nc.vector.tensor_mul(tm[:], ohw[:], io_f[:])
                fwf = wk.tile([128, 1], F32, tag="fwf", name="fwf")
                nc.vector.tensor_reduce(fwf[:], tm[:], axis=mybir.AxisListType.X,
                                        op=ALU.add)
                flf = wk.tile([128, 1], F32, tag="flf", name="flf")
                nc.vector.tensor_scalar(flf[:], fwf[:], -100.0, qf,
                                        op0=ALU.mult, op1=ALU.add)
                ohl = wk.tile([128, 100], F32, tag="ohl", name="ohl", bufs=2)
                nc.vector.tensor_scalar(ohl[:], io_f[:], flf[:], None,
                                        op0=ALU.is_equal)
                return ohw, ohl

            def trans_oh(bc, ohw, ohl):
                """transpose one-hots into [100, BS] bf16 table operands."""
                bs = bsl(bc)
                pw = p3.tile([128, 1024], F32, tag="fc3p", name="ptw")
                nc.tensor.transpose(pw[0:100, 0:128], ohw[:], ident[:])
                nc.vector.tensor_copy(ohwT[:, bs], pw[0:100, 0:128])
                nc.tensor.transpose(pw[0:100, 512:640], ohl[:], ident[:])
                nc.vector.tensor_copy(ohlT[:, bs], pw[0:100, 512:640])

            # ================= main loop (software-pipelined) =============
            # LSTM of step 0 (x-path prologue)
            for bc in range(2):
                gp = gates_layer1(bc, 0)
                if dbg and bc == 0:
                    g1c = wk.tile([128, 8, 128], F32, tag="g1c", name="g1c")
                    nc.vector.tensor_copy(g1c[:], gp[:])
                    nc.sync.dma_start(dbg_g1[:], g1c[:])
                a = gate_acts(bc, gp, b1r)
                cell_update(bc, a, c1_t, h1_t, h1h, h1l)
                if dbg and bc == 0:
                    nc.sync.dma_start(dbg_h1[:], h1_t[:])
                    nc.sync.dma_start(dbg_c1[:], c1_t[:])
            for bc in range(2):
                a = gate_acts(bc, gates_layer2(bc), b2r)
                cell_update(bc, a, c2_t, h2_t, h2h, h2l)

            for t in range(delta):
                last = (t == delta - 1)
                cv0, ci0 = fc3_scan(0)
                m0 = merge_onehot(0, t, cv0, ci0)
                cv1, ci1 = fc3_scan(1)
                if not last:
                    trans_oh(0, m0[0], m0[1])
                    gp1a = gates_layer1(0, t + 1)
                    aa = gate_acts(0, gp1a, b1rf)
                m1 = merge_onehot(1, t, cv1, ci1)
                if not last:
                    trans_oh(1, m1[0], m1[1])
                    gp1b = gates_layer1(1, t + 1)
                    ab = gate_acts(1, gp1b, b1rf)
                    cell_update(0, aa, c1_t, h1_t, h1h, h1l)
                    cell_update(1, ab, c1_t, h1_t, h1h, h1l)
                    a2a = gate_acts(0, gates_layer2(0), b2r)
                    cell_update(0, a2a, c2_t, h2_t, h2h, h2l)
                    a2b = gate_acts(1, gates_layer2(1), b2r)
                    cell_update(1, a2b, c2_t, h2_t, h2h, h2l)

            for bc in range(2):
                nc.sync.dma_start(idx_out[bc], outi[:, bc, :])
    nc.finalize()
    return nc


def _prep_shared(inputs):
    f32, f64 = np.float32, np.float64
    bf = ml_dtypes.bfloat16

    def split(a):
        ah = a.astype(bf)
        al = (a.astype(f32) - ah.astype(f32)).astype(bf)
        return ah, al

    def fmT(w):  # [out, in] -> lhsT chunks [2, 128, out]
        wt = np.ascontiguousarray(w.T.astype(f32))
        return wt.reshape(2, 128, wt.shape[1])

    W = {k: np.asarray(v) for k, v in inputs.items()}
    fc1, fc2, fc3 = (W['fc1_W'].astype(f64), W['fc2_W'].astype(f64),
                     W['fc3_W'].astype(f64))
    W3f = (fc3 @ fc2 @ fc1).astype(f32)                       # [Q, 256]
    b3f = (W['fc3_b'].astype(f64) + fc3 @ W['fc2_b'].astype(f64)
           + (fc3 @ fc2) @ W['fc1_b'].astype(f64)).astype(f32)
    Aw = (W['lstm1_Wih'][:, :128].astype(f64)
          @ W['fcqw_W'].astype(f64)).astype(f32)              # [1024, 100]
    Al = (W['lstm1_Wih'][:, 128:].astype(f64)
          @ W['fcql_W'].astype(f64)).astype(f32)
    embb = np.concatenate([W['fcqw_b'], W['fcql_b']]).astype(f64)
    b1f = (W['lstm1_b'].astype(f64)
           + W['lstm1_Wih'].astype(f64) @ embb).astype(f32)

    shared = {}
    for name, w in (("w1ih", W['lstm1_Wih']), ("w1hh", W['lstm1_Whh']),
                    ("w2ih", W['lstm2_Wih']), ("w2hh", W['lstm2_Whh'])):
        h_, l_ = split(fmT(w))
        shared[name + "Th"] = h_
        shared[name + "Tl"] = l_
    shared["w3Th"], shared["w3Tl"] = split(fmT(W3f))
    shared["awTh"], shared["awTl"] = split(np.ascontiguousarray(Aw.T))
    shared["alTh"], shared["alTl"] = split(np.ascontiguousarray(Al.T))
    shared["b1r"] = W['lstm1_b'].astype(f32).reshape(8, 128).T.copy()
    shared["b1rf"] = b1f.reshape(8, 128).T.copy()
    shared["b2r"] = W['lstm2_b'].astype(f32).reshape(8, 128).T.copy()
    # fc3 bias as 3 bf16 terms (seeded into PSUM via a K=3 ones matmul)
    b3a = b3f.astype(bf)
    r1 = (b3f - b3a.astype(f32)).astype(f32)
    b3b = r1.astype(bf)
    b3c = (r1 - b3b.astype(f32)).astype(bf)
    shared["b3t"] = np.ascontiguousarray(np.stack([b3a, b3b, b3c]))
    # per-candidate-slot global index base: slot 8g+k -> 1024g
    iob = np.repeat(np.arange(NG, dtype=f32) * 1024.0, 8)
    shared["iob80"] = np.ascontiguousarray(np.broadcast_to(iob, (128, 80)))
    return shared


def _per_core(inputs, c):
    f32 = np.float32
    bf = ml_dtypes.bfloat16
    sl = slice(c * BS, (c + 1) * BS)

    def fmT(a):  # [BS, 256] -> [2, 128, BS]
        return np.ascontiguousarray(a.T.astype(f32)).reshape(2, 128, BS)

    def split(a):
        ah = a.astype(bf)
        al = (a - ah.astype(f32)).astype(bf)
        return ah, al

    x = fmT(np.asarray(inputs["x"])[sl, 0, :])
    h1 = fmT(np.asarray(inputs["h1"])[0, sl])
    h2 = fmT(np.asarray(inputs["h2"])[0, sl])
    xh, xl = split(x)
    h1h, h1l = split(h1)
    h2h, h2l = split(h2)
    return {
        "xh": xh, "xl": xl,
        "c1_fm": fmT(np.asarray(inputs["c1"])[0, sl]),
        "c2_fm": fmT(np.asarray(inputs["c2"])[0, sl]),
        "h1h": h1h, "h1l": h1l, "h2h": h2h, "h2l": h2l,
    }


def kernel(**inputs):
    key = "nc"
    if key not in _CACHE:
        _CACHE[key] = _build_nc()
    nc = _CACHE[key]

    shared = _prep_shared(inputs)
    in_maps = []
    for c in range(NCORES):
        m = dict(shared)
        m.update(_per_core(inputs, c))
        in_maps.append(m)

    from concourse.bass_utils import run_bass_kernel_spmd
    res = run_bass_kernel_spmd(nc, in_maps, list(range(NCORES)))
    return assemble(res.results)


def assemble(results):
    traj = np.zeros((B, DELTA, K4, 2), np.float32)
    for c, r in enumerate(results):
        idx = r["idx_out"].reshape(2, 128, 20).astype(np.int64)
        for bc in range(2):
            rows = slice(c * BS + bc * 128, c * BS + (bc + 1) * 128)
            top4 = idx[bc, :, 0:4]
            traj[rows, 0, :, 0] = (top4 % QL).astype(np.float32)
            traj[rows, 0, :, 1] = (top4 // QL).astype(np.float32)
            greedy = idx[bc, :, 4:4 + DELTA - 1]
            traj[rows, 1:, 0, 0] = (greedy % QL).astype(np.float32)
            traj[rows, 1:, 0, 1] = (greedy // QL).astype(np.float32)
    return traj


# revision 50
# speedup vs baseline: 1.2548x; 1.0003x over previous
"""Trainium2 Bass kernel for nn_Decoder (2-layer LSTM + 3 FC + top-k decode).

Strategy: pure data parallelism over batch (2048 -> 8 cores x 256).
Feature-major activations [feat, batch]. All matmuls are 3-term bf16
splits (hi/lo), empirically exact for every argmax decision. fc1/fc2/fc3
fold on the host into one 256->10000 matmul (fp64 compose). For steps
>= 1 the LSTM1 input matmul becomes one-hot table matmuls
(tables = W1ih @ fcq{w,l}_W, host fp64). The fc3 bias is seeded into
PSUM by a K=3 ones-matmul of a bf16 bias triple, and the top-k scan
(max8 + find_index8) reads PSUM directly - logits never touch SBUF.
The decode pipeline is split into two 128-row chunks and
software-pipelined: the next step's LSTM work is emitted between this
step's per-chunk merge/one-hot phases so PE never drains.
"""
import numpy as np
import ml_dtypes

B, D, H = 2048, 256, 256
K4, QW, QL, DELTA = 4, 100, 100, 16
Q = QW * QL
NCORES = 8
BS = B // NCORES          # 256 rows per core
NG = 10                   # psum scan groups of 2 fc3 tiles
G4 = 4 * H                # 1024 gates

_CACHE = {}


def _build_nc(delta=DELTA, dbg=False):
    import concourse.mybir as mybir
    import concourse.tile as tile
    import concourse.bacc as bacc
    from concourse.masks import make_identity

    F32 = mybir.dt.float32
    BF16 = mybir.dt.bfloat16
    U32 = mybir.dt.uint32
    AF = mybir.ActivationFunctionType
    ALU = mybir.AluOpType

    nc = bacc.Bacc(None, target_bir_lowering=False, debug=False)

    def din(name, shape, dt=F32):
        return nc.dram_tensor(name, shape, dt, kind="ExternalInput")

    # per-core inputs
    xh_in = din("xh", [2, 128, BS], BF16)
    xl_in = din("xl", [2, 128, BS], BF16)
    c1_in = din("c1_fm", [2, 128, BS])
    c2_in = din("c2_fm", [2, 128, BS])
    h1h_in = din("h1h", [2, 128, BS], BF16)
    h1l_in = din("h1l", [2, 128, BS], BF16)
    h2h_in = din("h2h", [2, 128, BS], BF16)
    h2l_in = din("h2l", [2, 128, BS], BF16)
    # shared weights (bf16 hi/lo pairs, lhsT layout)
    w1ihh_in = din("w1ihTh", [2, 128, G4], BF16)
    w1ihl_in = din("w1ihTl", [2, 128, G4], BF16)
    w1hhh_in = din("w1hhTh", [2, 128, G4], BF16)
    w1hhl_in = din("w1hhTl", [2, 128, G4], BF16)
    w2ihh_in = din("w2ihTh", [2, 128, G4], BF16)
    w2ihl_in = din("w2ihTl", [2, 128, G4], BF16)
    w2hhh_in = din("w2hhTh", [2, 128, G4], BF16)
    w2hhl_in = din("w2hhTl", [2, 128, G4], BF16)
    w3h_in = din("w3Th", [2, 128, Q], BF16)
    w3l_in = din("w3Tl", [2, 128, Q], BF16)
    awh_in = din("awTh", [100, G4], BF16)
    awl_in = din("awTl", [100, G4], BF16)
    alh_in = din("alTh", [100, G4], BF16)
    all_in = din("alTl", [100, G4], BF16)
    b1r_in = din("b1r", [128, 8])
    b1rf_in = din("b1rf", [128, 8])
    b2r_in = din("b2r", [128, 8])
    b3t_in = din("b3t", [3, Q], mybir.dt.bfloat16)
    iob80_in = din("iob80", [128, 80])

    idx_out = nc.dram_tensor("idx_out", [2, 128, 20], U32, kind="ExternalOutput")
    if dbg:
        dbg_lq = nc.dram_tensor("dbg_lq", [128, 1024], F32, kind="ExternalOutput")
        dbg_cv = nc.dram_tensor("dbg_cv", [128, 80], F32, kind="ExternalOutput")
        dbg_ci = nc.dram_tensor("dbg_ci", [128, 80], F32, kind="ExternalOutput")
        dbg_h2 = nc.dram_tensor("dbg_h2", [128, 2, BS], F32, kind="ExternalOutput")
        dbg_h1 = nc.dram_tensor("dbg_h1", [128, 2, BS], F32, kind="ExternalOutput")
        dbg_c1 = nc.dram_tensor("dbg_c1", [128, 2, BS], F32, kind="ExternalOutput")
        dbg_g1 = nc.dram_tensor("dbg_g1", [128, 8, 128], F32, kind="ExternalOutput")

    with tile.TileContext(nc) as tc:
        with (
            tc.tile_pool(name="wp", bufs=1) as wp,
            tc.tile_pool(name="st", bufs=1) as st,
            tc.tile_pool(name="wk", bufs=2) as wk,
            tc.tile_pool(name="p3", bufs=2, space="PSUM") as p3,
            tc.tile_pool(name="pg", bufs=2, space="PSUM") as pg,
        ):
            # ---- weight / const loads (ordered by first use) ----
            def wload(src, shape, tag, dt=F32):
                t = wp.tile(shape, dt, tag=tag, name=tag)
                if len(shape) == 3 and shape[1] == 2:
                    nc.sync.dma_start(t[:], src[:].rearrange("c p f -> p c f"))
                else:
                    nc.sync.dma_start(t[:], src[:])
                return t

            w1ihh = wload(w1ihh_in, [128, 2, G4], "w1ihh", BF16)
            w1ihl = wload(w1ihl_in, [128, 2, G4], "w1ihl", BF16)
            w1hhh = wload(w1hhh_in, [128, 2, G4], "w1hhh", BF16)
            w1hhl = wload(w1hhl_in, [128, 2, G4], "w1hhl", BF16)
            b1r = wload(b1r_in, [128, 8], "b1r")
            b2r = wload(b2r_in, [128, 8], "b2r")

            one3 = wp.tile([3, 128], BF16)
            nc.vector.memset(one3[:], 1.0)
            ident = wp.tile([128, 128], F32)
            make_identity(nc, ident[:])
            io_f = wp.tile([128, 100], F32)
            nc.gpsimd.iota(io_f[:], pattern=[[1, 100]], base=0,
                           channel_multiplier=0,
                           allow_small_or_imprecise_dtypes=True)
            io100 = wp.tile([128, 100], F32)
            nc.gpsimd.iota(io100[:], pattern=[[100, 100]], base=0,
                           channel_multiplier=0,
                           allow_small_or_imprecise_dtypes=True)
            io80 = wp.tile([128, 80], F32)
            nc.gpsimd.iota(io80[:], pattern=[[1, 80]], base=0,
                           channel_multiplier=0,
                           allow_small_or_imprecise_dtypes=True)

            # ---- persistent state ----
            def sload(src, tag, dt=F32):
                t = st.tile([128, 2, BS], dt, tag=tag, name=tag)
                nc.sync.dma_start(t[:], src[:].rearrange("c p b -> p c b"))
                return t

            xh = sload(xh_in, "xh", BF16)
            xl = sload(xl_in, "xl", BF16)
            c1_t = sload(c1_in, "c1")
            c2_t = sload(c2_in, "c2")
            h1h = sload(h1h_in, "h1h", BF16)
            h1l = sload(h1l_in, "h1l", BF16)
            h2h = sload(h2h_in, "h2h", BF16)
            h2l = sload(h2l_in, "h2l", BF16)
            # bulk weights after the step-0 dependencies
            w2ihh = wload(w2ihh_in, [128, 2, G4], "w2ihh", BF16)
            w2ihl = wload(w2ihl_in, [128, 2, G4], "w2ihl", BF16)
            w2hhh = wload(w2hhh_in, [128, 2, G4], "w2hhh", BF16)
            w2hhl = wload(w2hhl_in, [128, 2, G4], "w2hhl", BF16)
            b3t = wload(b3t_in, [3, Q], "b3t", BF16)
            w3h = wload(w3h_in, [128, 2, Q], "w3h", BF16)
            w3l = wload(w3l_in, [128, 2, Q], "w3l", BF16)
            iob80 = wload(iob80_in, [128, 80], "iob80")
            awh = wload(awh_in, [100, G4], "awh", BF16)
            awl = wload(awl_in, [100, G4], "awl", BF16)
            alh = wload(alh_in, [100, G4], "alh", BF16)
            all_ = wload(all_in, [100, G4], "all", BF16)
            b1rf = wload(b1rf_in, [128, 8], "b1rf")
            h1_t = st.tile([128, 2, BS], F32, tag="h1", name="h1")
            h2_t = st.tile([128, 2, BS], F32, tag="h2", name="h2")
            ohwT = st.tile([100, BS], BF16, tag="ohwT", name="ohwT")
            ohlT = st.tile([100, BS], BF16, tag="ohlT", name="ohlT")
            outi = st.tile([128, 2, 20], U32, tag="outi", name="outi")
            nc.vector.memset(outi[:], 0)

            def bsl(bc):
                return slice(128 * bc, 128 * (bc + 1))

            # ---- per-chunk LSTM matmul phases ----
            def gates_layer1(bc, t):
                """gates1 psum: recurrent + x (t=0) / one-hot table part."""
                gp = pg.tile([128, 8, 128], F32, tag="g1", name="g1")
                bs = bsl(bc)
                for g in range(8):
                    sl = slice(128 * g, 128 * (g + 1))
                    o = gp[:, g, :]
                    for k in range(2):
                        nc.tensor.matmul(o, w1hhh[:, k, sl], h1h[:, k, bs],
                                         start=(k == 0), stop=False)
                        nc.tensor.matmul(o, w1hhh[:, k, sl], h1l[:, k, bs],
                                         start=False, stop=False)
                        nc.tensor.matmul(o, w1hhl[:, k, sl], h1h[:, k, bs],
                                         start=False, stop=False)
                    if t == 0:
                        for k in range(2):
                            nc.tensor.matmul(o, w1ihh[:, k, sl], xh[:, k, bs],
                                             start=False, stop=False)
                            nc.tensor.matmul(o, w1ihh[:, k, sl], xl[:, k, bs],
                                             start=False, stop=False)
                            nc.tensor.matmul(o, w1ihl[:, k, sl], xh[:, k, bs],
                                             start=False, stop=(k == 1))
                    else:
                        nc.tensor.matmul(o, awh[:, sl], ohwT[:, bs],
                                         start=False, stop=False)
                        nc.tensor.matmul(o, awl[:, sl], ohwT[:, bs],
                                         start=False, stop=False)
                        nc.tensor.matmul(o, alh[:, sl], ohlT[:, bs],
                                         start=False, stop=False)
                        nc.tensor.matmul(o, all_[:, sl], ohlT[:, bs],
                                         start=False, stop=True)
                return gp

            def gates_layer2(bc):
                gp = pg.tile([128, 8, 128], F32, tag="g1", name="g2")
                bs = bsl(bc)
                for g in range(8):
                    sl = slice(128 * g, 128 * (g + 1))
                    o = gp[:, g, :]
                    for k in range(2):
                        nc.tensor.matmul(o, w2ihh[:, k, sl], h1h[:, k, bs],
                                         start=(k == 0), stop=False)
                        nc.tensor.matmul(o, w2ihh[:, k, sl], h1l[:, k, bs],
                                         start=False, stop=False)
                        nc.tensor.matmul(o, w2ihl[:, k, sl], h1h[:, k, bs],
                                         start=False, stop=False)
                    for k in range(2):
                        nc.tensor.matmul(o, w2hhh[:, k, sl], h2h[:, k, bs],
                                         start=False, stop=False)
                        nc.tensor.matmul(o, w2hhh[:, k, sl], h2l[:, k, bs],
                                         start=False, stop=False)
                        nc.tensor.matmul(o, w2hhl[:, k, sl], h2h[:, k, bs],
                                         start=False, stop=(k == 1))
                return gp

            def gate_acts(bc, gp, br):
                """sigmoid/tanh activations with per-slice gate biases."""
                si = wk.tile([128, 2, 128], F32, tag="si")
                sf = wk.tile([128, 2, 128], F32, tag="sf")
                tg = wk.tile([128, 2, 128], F32, tag="tg")
                so = wk.tile([128, 2, 128], F32, tag="so")
                for ch in range(2):
                    nc.scalar.activation(si[:, ch, :], gp[:, 0 + ch, :],
                                         AF.Sigmoid, bias=br[:, 0 + ch:1 + ch])
                    nc.scalar.activation(sf[:, ch, :], gp[:, 2 + ch, :],
                                         AF.Sigmoid, bias=br[:, 2 + ch:3 + ch])
                    nc.scalar.activation(tg[:, ch, :], gp[:, 4 + ch, :],
                                         AF.Tanh, bias=br[:, 4 + ch:5 + ch])
                    nc.scalar.activation(so[:, ch, :], gp[:, 6 + ch, :],
                                         AF.Sigmoid, bias=br[:, 6 + ch:7 + ch])
                return si, sf, tg, so

            def cell_update(bc, acts, cT, hT, hh, hl):
                si, sf, tg, so = acts
                bs = bsl(bc)
                csl = cT[:, :, bs]
                hsl = hT[:, :, bs]
                t1 = wk.tile([128, 2, 128], F32, tag="t1", bufs=1)
                t2 = wk.tile([128, 2, 128], F32, tag="t2", bufs=1)
                nc.vector.tensor_mul(t1[:], sf[:], csl)
                nc.vector.tensor_mul(t2[:], si[:], tg[:])
                nc.vector.tensor_add(csl, t1[:], t2[:])
                t3 = wk.tile([128, 2, 128], F32, tag="t3", bufs=1)
                nc.scalar.activation(t3[:], csl, AF.Tanh)
                nc.vector.tensor_mul(hsl, so[:], t3[:])
                nc.vector.tensor_copy(hh[:, :, bs], hsl)
                nc.vector.tensor_sub(hl[:, :, bs], hsl, hh[:, :, bs])

            # ---- fc3 + PSUM-direct scan for one chunk ----
            # 20 tiles: 19 x 512 + 1 x 272; scan groups of 2 tiles in one
            # flat 2-bank psum tile so indices stay affine (base 1024*gi)
            FTILES = [(i * 512, 512) for i in range(19)] + [(9728, 272)]

            def fc3_scan(bc):
                bs = bsl(bc)
                cand_v = wk.tile([128, 80], F32, tag="candv", name="candv")
                if not hasattr(fc3_scan, "ran"):
                    fc3_scan.ran = [False]
                cand_iu = wk.tile([128, 80], U32, tag="candiu", name="candiu")
                stats = [(h2h[:, 0, bs], w3h[:, 0, :]),
                         (h2h[:, 1, bs], w3h[:, 1, :]),
                         (h2l[:, 0, bs], w3h[:, 0, :]),
                         (h2l[:, 1, bs], w3h[:, 1, :]),
                         (h2h[:, 0, bs], w3l[:, 0, :]),
                         (h2h[:, 1, bs], w3l[:, 1, :])]
                for gi in range(NG):
                    pt = p3.tile([128, 1024], F32, tag="fc3p", name="fc3p")
                    spanw = 0
                    for ti in range(2):
                        n0, wdt = FTILES[2 * gi + ti]
                        o = pt[:, 512 * ti:512 * ti + wdt]
                        # seed PSUM with the fc3 bias triple, then accumulate
                        nc.tensor.matmul(o, one3[:], b3t[:, n0:n0 + wdt],
                                         start=True, stop=False)
                        for j, (stat, w) in enumerate(stats):
                            nc.tensor.matmul(o, stat, w[:, n0:n0 + wdt],
                                             start=False, stop=(j == 5))
                        spanw = 512 * ti + wdt
                    span = pt[:, 0:spanw]
                    if dbg and bc == 0 and gi == 0 and not fc3_scan.ran[0]:
                        lqg = wk.tile([128, 1024], F32, tag="lqg", name="lqg")
                        nc.vector.tensor_copy(lqg[:, 0:spanw], span)
                        nc.sync.dma_start(dbg_lq[:], lqg[:])
                    nc.vector.max(cand_v[:, 8 * gi:8 * gi + 8], span)
                    nc.vector.max_index(cand_iu[:, 8 * gi:8 * gi + 8],
                                        cand_v[:, 8 * gi:8 * gi + 8], span)
                cif = wk.tile([128, 80], F32, tag="cif", name="cif")
                nc.vector.tensor_copy(cif[:], cand_iu[:])
                cand_i = wk.tile([128, 80], F32, tag="candi", name="candi")
                nc.vector.tensor_add(cand_i[:], cif[:], iob80[:])
                if dbg and bc == 0 and not fc3_scan.ran[0]:
                    nc.sync.dma_start(dbg_cv[:], cand_v[:])
                    nc.sync.dma_start(dbg_ci[:], cand_i[:])
                    nc.sync.dma_start(dbg_h2[:], h2_t[:])
                    fc3_scan.ran[0] = True
                return cand_v, cand_i

            def merge_onehot(bc, t, cand_v, cand_i):
                """top-k merge, trajectory index write, one-hot build."""
                vm8 = wk.tile([128, 8], F32, tag="vm8", name="vm8")
                pm8 = wk.tile([128, 8], U32, tag="pm8", name="pm8")
                nc.vector.max(vm8[:], cand_v[:])
                nc.vector.max_index(pm8[:], vm8[:], cand_v[:])
                pmf = wk.tile([128, 8], F32, tag="pmf", name="pmf")
                nc.vector.tensor_copy(pmf[:], pm8[:])
                nk = 4 if t == 0 else 1
                qsel = wk.tile([128, 4], F32, tag="qsel", name="qsel")
                for kk in range(nk):
                    ohp = wk.tile([128, 80], F32, tag="ohp", name="ohp")
                    nc.vector.tensor_scalar(ohp[:], io80[:], pmf[:, kk:kk + 1],
                                            None, op0=ALU.is_equal)
                    tmq = wk.tile([128, 80], F32, tag="tmq", name="tmq")
                    nc.vector.tensor_mul(tmq[:], ohp[:], cand_i[:])
                    nc.vector.tensor_reduce(qsel[:, kk:kk + 1], tmq[:],
                                            axis=mybir.AxisListType.X,
                                            op=ALU.add)
                if t == 0:
                    nc.vector.tensor_copy(outi[:, bc, 0:4], qsel[:, 0:4])
                else:
                    nc.vector.tensor_copy(outi[:, bc, 4 + t - 1:5 + t - 1],
                                          qsel[:, 0:1])
                if t == delta - 1:
                    return None, None
                qf = qsel[:, 0:1]
                m_ge = wk.tile([128, 100], F32, tag="mge", name="mge", bufs=1)
                nc.vector.tensor_scalar(m_ge[:], io100[:], qf, None,
                                        op0=ALU.is_le)
                qm = wk.tile([128, 1], F32, tag="qm", name="qm")
                nc.vector.tensor_scalar(qm[:], qf, -100.0, None, op0=ALU.add)
                m_lt = wk.tile([128, 100], F32, tag="mlt", name="mlt", bufs=1)
                nc.vector.tensor_scalar(m_lt[:], io100[:], qm[:], None,
                                        op0=ALU.is_gt)
                ohw = wk.tile([128, 100], F32, tag="ohw", name="ohw", bufs=2)
                nc.vector.tensor_mul(ohw[:], m_ge[:], m_lt[:])
                tm = wk.tile([128, 100], F32, tag="tm", name="tm", bufs=1)
                nc.vector.tensor_mul(tm[:], ohw[:], io_f[:])
                fwf = wk.tile([128, 1], F32, tag="fwf", name="fwf")
                nc.vector.tensor_reduce(fwf[:], tm[:], axis=mybir.AxisListType.X,
                                        op=ALU.add)
                flf = wk.tile([128, 1], F32, tag="flf", name="flf")
                nc.vector.tensor_scalar(flf[:], fwf[:], -100.0, qf,
                                        op0=ALU.mult, op1=ALU.add)
                ohl = wk.tile([128, 100], F32, tag="ohl", name="ohl", bufs=2)
                nc.vector.tensor_scalar(ohl[:], io_f[:], flf[:], None,
                                        op0=ALU.is_equal)
                return ohw, ohl

            def trans_oh(bc, ohw, ohl):
                """transpose one-hots into [100, BS] bf16 table operands."""
                bs = bsl(bc)
                pw = p3.tile([128, 1024], F32, tag="fc3p", name="ptw")
                nc.tensor.transpose(pw[0:100, 0:128], ohw[:], ident[:])
                nc.vector.tensor_copy(ohwT[:, bs], pw[0:100, 0:128])
                nc.tensor.transpose(pw[0:100, 512:640], ohl[:], ident[:])
                nc.vector.tensor_copy(ohlT[:, bs], pw[0:100, 512:640])

            # ================= main loop (software-pipelined) =============
            # LSTM of step 0 (x-path prologue)
            for bc in range(2):
                gp = gates_layer1(bc, 0)
                if dbg and bc == 0:
                    g1c = wk.tile([128, 8, 128], F32, tag="g1c", name="g1c")
                    nc.vector.tensor_copy(g1c[:], gp[:])
                    nc.sync.dma_start(dbg_g1[:], g1c[:])
                a = gate_acts(bc, gp, b1r)
                cell_update(bc, a, c1_t, h1_t, h1h, h1l)
                if dbg and bc == 0:
                    nc.sync.dma_start(dbg_h1[:], h1_t[:])
                    nc.sync.dma_start(dbg_c1[:], c1_t[:])
            for bc in range(2):
                a = gate_acts(bc, gates_layer2(bc), b2r)
                cell_update(bc, a, c2_t, h2_t, h2h, h2l)

            for t in range(delta):
                last = (t == delta - 1)
                cv0, ci0 = fc3_scan(0)
                m0 = merge_onehot(0, t, cv0, ci0)
                cv1, ci1 = fc3_scan(1)
                if not last:
                    trans_oh(0, m0[0], m0[1])
                    gp1a = gates_layer1(0, t + 1)
                    aa = gate_acts(0, gp1a, b1rf)
                m1 = merge_onehot(1, t, cv1, ci1)
                if not last:
                    trans_oh(1, m1[0], m1[1])
                    gp1b = gates_layer1(1, t + 1)
                    ab = gate_acts(1, gp1b, b1rf)
                    cell_update(0, aa, c1_t, h1_t, h1h, h1l)
                    cell_update(1, ab, c1_t, h1_t, h1h, h1l)
                    a2a = gate_acts(0, gates_layer2(0), b2r)
                    cell_update(0, a2a, c2_t, h2_t, h2h, h2l)
                    a2b = gate_acts(1, gates_layer2(1), b2r)
                    cell_update(1, a2b, c2_t, h2_t, h2h, h2l)

            for bc in range(2):
                nc.sync.dma_start(idx_out[bc], outi[:, bc, :])
    nc.finalize()
    return nc


def _prep_shared(inputs):
    f32, f64 = np.float32, np.float64
    bf = ml_dtypes.bfloat16

    def split(a):
        ah = a.astype(bf)
        al = (a.astype(f32) - ah.astype(f32)).astype(bf)
        return ah, al

    def fmT(w):  # [out, in] -> lhsT chunks [2, 128, out]
        wt = np.ascontiguousarray(w.T.astype(f32))
        return wt.reshape(2, 128, wt.shape[1])

    W = {k: np.asarray(v) for k, v in inputs.items()}
    fc1, fc2, fc3 = (W['fc1_W'].astype(f64), W['fc2_W'].astype(f64),
                     W['fc3_W'].astype(f64))
    W3f = (fc3 @ fc2 @ fc1).astype(f32)                       # [Q, 256]
    b3f = (W['fc3_b'].astype(f64) + fc3 @ W['fc2_b'].astype(f64)
           + (fc3 @ fc2) @ W['fc1_b'].astype(f64)).astype(f32)
    Aw = (W['lstm1_Wih'][:, :128].astype(f64)
          @ W['fcqw_W'].astype(f64)).astype(f32)              # [1024, 100]
    Al = (W['lstm1_Wih'][:, 128:].astype(f64)
          @ W['fcql_W'].astype(f64)).astype(f32)
    embb = np.concatenate([W['fcqw_b'], W['fcql_b']]).astype(f64)
    b1f = (W['lstm1_b'].astype(f64)
           + W['lstm1_Wih'].astype(f64) @ embb).astype(f32)

    shared = {}
    for name, w in (("w1ih", W['lstm1_Wih']), ("w1hh", W['lstm1_Whh']),
                    ("w2ih", W['lstm2_Wih']), ("w2hh", W['lstm2_Whh'])):
        h_, l_ = split(fmT(w))
        shared[name + "Th"] = h_
        shared[name + "Tl"] = l_
    shared["w3Th"], shared["w3Tl"] = split(fmT(W3f))
    shared["awTh"], shared["awTl"] = split(np.ascontiguousarray(Aw.T))
    shared["alTh"], shared["alTl"] = split(np.ascontiguousarray(Al.T))
    shared["b1r"] = W['lstm1_b'].astype(f32).reshape(8, 128).T.copy()
    shared["b1rf"] = b1f.reshape(8, 128).T.copy()
    shared["b2r"] = W['lstm2_b'].astype(f32).reshape(8, 128).T.copy()
    # fc3 bias as 3 bf16 terms (seeded into PSUM via a K=3 ones matmul)
    b3a = b3f.astype(bf)
    r1 = (b3f - b3a.astype(f32)).astype(f32)
    b3b = r1.astype(bf)
    b3c = (r1 - b3b.astype(f32)).astype(bf)
    shared["b3t"] = np.ascontiguousarray(np.stack([b3a, b3b, b3c]))
    # per-candidate-slot global index base: slot 8g+k -> 1024g
    iob = np.repeat(np.arange(NG, dtype=f32) * 1024.0, 8)
    shared["iob80"] = np.ascontiguousarray(np.broadcast_to(iob, (128, 80)))
    return shared


def _per_core(inputs, c):
    f32 = np.float32
    bf = ml_dtypes.bfloat16
    sl = slice(c * BS, (c + 1) * BS)

    def fmT(a):  # [BS, 256] -> [2, 128, BS]
        return np.ascontiguousarray(a.T.astype(f32)).reshape(2, 128, BS)

    def split(a):
        ah = a.astype(bf)
        al = (a - ah.astype(f32)).astype(bf)
        return ah, al

    x = fmT(np.asarray(inputs["x"])[sl, 0, :])
    h1 = fmT(np.asarray(inputs["h1"])[0, sl])
    h2 = fmT(np.asarray(inputs["h2"])[0, sl])
    xh, xl = split(x)
    h1h, h1l = split(h1)
    h2h, h2l = split(h2)
    return {
        "xh": xh, "xl": xl,
        "c1_fm": fmT(np.asarray(inputs["c1"])[0, sl]),
        "c2_fm": fmT(np.asarray(inputs["c2"])[0, sl]),
        "h1h": h1h, "h1l": h1l, "h2h": h2h, "h2l": h2l,
    }


def kernel(**inputs):
    key = "nc"
    if key not in _CACHE:
        _CACHE[key] = _build_nc()
    nc = _CACHE[key]

    shared = _prep_shared(inputs)
    in_maps = []
    for c in range(NCORES):
        m = dict(shared)
        m.update(_per_core(inputs, c))
        in_maps.append(m)

    from concourse.bass_utils import run_bass_kernel_spmd
    res = run_bass_kernel_spmd(nc, in_maps, list(range(NCORES)))
    return assemble(res.results)


def assemble(results):
    traj = np.zeros((B, DELTA, K4, 2), np.float32)
    for c, r in enumerate(results):
        idx = r["idx_out"].reshape(2, 128, 20).astype(np.int64)
        for bc in range(2):
            rows = slice(c * BS + bc * 128, c * BS + (bc + 1) * 128)
            top4 = idx[bc, :, 0:4]
            traj[rows, 0, :, 0] = (top4 % QL).astype(np.float32)
            traj[rows, 0, :, 1] = (top4 // QL).astype(np.float32)
            greedy = idx[bc, :, 4:4 + DELTA - 1]
            traj[rows, 1:, 0, 0] = (greedy % QL).astype(np.float32)
            traj[rows, 1:, 0, 1] = (greedy // QL).astype(np.float32)
    return traj


# revision 51
# speedup vs baseline: 1.2662x; 1.0091x over previous
"""Trainium2 Bass kernel for nn_Decoder (2-layer LSTM + 3 FC + top-k decode).

Strategy: pure data parallelism over batch (2048 -> 8 cores x 256).
Feature-major activations [feat, batch]. All matmuls are 3-term bf16
splits (hi/lo), empirically exact for every argmax decision. fc1/fc2/fc3
fold on the host into one 256->10000 matmul (fp64 compose). For steps
>= 1 the LSTM1 input matmul becomes one-hot table matmuls
(tables = W1ih @ fcq{w,l}_W, host fp64). The fc3 bias is seeded into
PSUM by a K=3 ones-matmul of a bf16 bias triple, and the top-k scan
(max8 + find_index8) reads PSUM directly - logits never touch SBUF.
The decode pipeline is split into two 128-row chunks and
software-pipelined: the next step's LSTM work is emitted between this
step's per-chunk merge/one-hot phases so PE never drains.
"""
import numpy as np
import ml_dtypes

B, D, H = 2048, 256, 256
K4, QW, QL, DELTA = 4, 100, 100, 16
Q = QW * QL
NCORES = 8
BS = B // NCORES          # 256 rows per core
NG = 10                   # psum scan groups of 2 fc3 tiles
G4 = 4 * H                # 1024 gates

_CACHE = {}


def _build_nc(delta=DELTA, dbg=False):
    import concourse.mybir as mybir
    import concourse.tile as tile
    import concourse.bacc as bacc
    from concourse.masks import make_identity

    F32 = mybir.dt.float32
    BF16 = mybir.dt.bfloat16
    U32 = mybir.dt.uint32
    AF = mybir.ActivationFunctionType
    ALU = mybir.AluOpType

    nc = bacc.Bacc(None, target_bir_lowering=False, debug=False)

    def din(name, shape, dt=F32):
        return nc.dram_tensor(name, shape, dt, kind="ExternalInput")

    # per-core inputs
    xh_in = din("xh", [2, 128, BS], BF16)
    xl_in = din("xl", [2, 128, BS], BF16)
    c1_in = din("c1_fm", [2, 128, BS])
    c2_in = din("c2_fm", [2, 128, BS])
    h1h_in = din("h1h", [2, 128, BS], BF16)
    h1l_in = din("h1l", [2, 128, BS], BF16)
    h2h_in = din("h2h", [2, 128, BS], BF16)
    h2l_in = din("h2l", [2, 128, BS], BF16)
    # shared weights (bf16 hi/lo pairs, lhsT layout)
    w1ihh_in = din("w1ihTh", [2, 128, G4], BF16)
    w1ihl_in = din("w1ihTl", [2, 128, G4], BF16)
    w1hhh_in = din("w1hhTh", [2, 128, G4], BF16)
    w1hhl_in = din("w1hhTl", [2, 128, G4], BF16)
    w2ihh_in = din("w2ihTh", [2, 128, G4], BF16)
    w2ihl_in = din("w2ihTl", [2, 128, G4], BF16)
    w2hhh_in = din("w2hhTh", [2, 128, G4], BF16)
    w2hhl_in = din("w2hhTl", [2, 128, G4], BF16)
    w3h_in = din("w3Th", [2, 128, Q], BF16)
    w3l_in = din("w3Tl", [2, 128, Q], BF16)
    awh_in = din("awTh", [100, G4], BF16)
    awl_in = din("awTl", [100, G4], BF16)
    alh_in = din("alTh", [100, G4], BF16)
    all_in = din("alTl", [100, G4], BF16)
    b1r_in = din("b1r", [128, 8])
    b1rf_in = din("b1rf", [128, 8])
    b2r_in = din("b2r", [128, 8])
    b3t_in = din("b3t", [3, Q], mybir.dt.bfloat16)
    iob80_in = din("iob80", [128, 80])

    idx_out = nc.dram_tensor("idx_out", [2, 128, 20], U32, kind="ExternalOutput")
    if dbg:
        dbg_lq = nc.dram_tensor("dbg_lq", [128, 1024], F32, kind="ExternalOutput")
        dbg_cv = nc.dram_tensor("dbg_cv", [128, 80], F32, kind="ExternalOutput")
        dbg_ci = nc.dram_tensor("dbg_ci", [128, 80], F32, kind="ExternalOutput")
        dbg_h2 = nc.dram_tensor("dbg_h2", [128, 2, BS], F32, kind="ExternalOutput")
        dbg_h1 = nc.dram_tensor("dbg_h1", [128, 2, BS], F32, kind="ExternalOutput")
        dbg_c1 = nc.dram_tensor("dbg_c1", [128, 2, BS], F32, kind="ExternalOutput")
        dbg_g1 = nc.dram_tensor("dbg_g1", [128, 8, 128], F32, kind="ExternalOutput")

    with tile.TileContext(nc) as tc:
        with (
            tc.tile_pool(name="wp", bufs=1) as wp,
            tc.tile_pool(name="st", bufs=1) as st,
            tc.tile_pool(name="wk", bufs=2) as wk,
            tc.tile_pool(name="p3", bufs=2, space="PSUM") as p3,
            tc.tile_pool(name="pg", bufs=2, space="PSUM") as pg,
        ):
            # ---- weight / const loads (ordered by first use) ----
            def wload(src, shape, tag, dt=F32):
                t = wp.tile(shape, dt, tag=tag, name=tag)
                if len(shape) == 3 and shape[1] == 2:
                    nc.sync.dma_start(t[:], src[:].rearrange("c p f -> p c f"))
                else:
                    nc.sync.dma_start(t[:], src[:])
                return t

            w1ihh = wload(w1ihh_in, [128, 2, G4], "w1ihh", BF16)
            w1ihl = wload(w1ihl_in, [128, 2, G4], "w1ihl", BF16)
            w1hhh = wload(w1hhh_in, [128, 2, G4], "w1hhh", BF16)
            w1hhl = wload(w1hhl_in, [128, 2, G4], "w1hhl", BF16)
            b1r = wload(b1r_in, [128, 8], "b1r")
            b2r = wload(b2r_in, [128, 8], "b2r")

            one3 = wp.tile([3, 128], BF16)
            nc.vector.memset(one3[:], 1.0)
            ident = wp.tile([128, 128], F32)
            make_identity(nc, ident[:])
            io_f = wp.tile([128, 100], F32)
            nc.gpsimd.iota(io_f[:], pattern=[[1, 100]], base=0,
                           channel_multiplier=0,
                           allow_small_or_imprecise_dtypes=True)
            io100 = wp.tile([128, 100], F32)
            nc.gpsimd.iota(io100[:], pattern=[[100, 100]], base=0,
                           channel_multiplier=0,
                           allow_small_or_imprecise_dtypes=True)
            io80 = wp.tile([128, 80], F32)
            nc.gpsimd.iota(io80[:], pattern=[[1, 80]], base=0,
                           channel_multiplier=0,
                           allow_small_or_imprecise_dtypes=True)

            # ---- persistent state ----
            def sload(src, tag, dt=F32):
                t = st.tile([128, 2, BS], dt, tag=tag, name=tag)
                nc.sync.dma_start(t[:], src[:].rearrange("c p b -> p c b"))
                return t

            xh = sload(xh_in, "xh", BF16)
            xl = sload(xl_in, "xl", BF16)
            c1_t = sload(c1_in, "c1")
            c2_t = sload(c2_in, "c2")
            h1h = sload(h1h_in, "h1h", BF16)
            h1l = sload(h1l_in, "h1l", BF16)
            h2h = sload(h2h_in, "h2h", BF16)
            h2l = sload(h2l_in, "h2l", BF16)
            # bulk weights after the step-0 dependencies
            w2ihh = wload(w2ihh_in, [128, 2, G4], "w2ihh", BF16)
            w2ihl = wload(w2ihl_in, [128, 2, G4], "w2ihl", BF16)
            w2hhh = wload(w2hhh_in, [128, 2, G4], "w2hhh", BF16)
            w2hhl = wload(w2hhl_in, [128, 2, G4], "w2hhl", BF16)
            b3t = wload(b3t_in, [3, Q], "b3t", BF16)
            # split the 5MB w3 loads so step-0's first fc3 groups can start
            # while the tail still streams in
            w3h = wp.tile([128, 2, Q], BF16, tag="w3h", name="w3h")
            w3l = wp.tile([128, 2, Q], BF16, tag="w3l", name="w3l")
            HQ = 5120
            for (tl, src) in ((w3h, w3h_in), (w3l, w3l_in)):
                nc.sync.dma_start(
                    tl[:, :, 0:HQ],
                    src[:, :, 0:HQ].rearrange("c p f -> p c f"))
            for (tl, src) in ((w3h, w3h_in), (w3l, w3l_in)):
                nc.sync.dma_start(
                    tl[:, :, HQ:Q],
                    src[:, :, HQ:Q].rearrange("c p f -> p c f"))
            iob80 = wload(iob80_in, [128, 80], "iob80")
            awh = wload(awh_in, [100, G4], "awh", BF16)
            awl = wload(awl_in, [100, G4], "awl", BF16)
            alh = wload(alh_in, [100, G4], "alh", BF16)
            all_ = wload(all_in, [100, G4], "all", BF16)
            b1rf = wload(b1rf_in, [128, 8], "b1rf")
            h1_t = st.tile([128, 2, BS], F32, tag="h1", name="h1")
            h2_t = st.tile([128, 2, BS], F32, tag="h2", name="h2")
            ohwT = st.tile([100, BS], BF16, tag="ohwT", name="ohwT")
            ohlT = st.tile([100, BS], BF16, tag="ohlT", name="ohlT")
            outi = st.tile([128, 2, 20], U32, tag="outi", name="outi")
            nc.vector.memset(outi[:], 0)

            def bsl(bc):
                return slice(128 * bc, 128 * (bc + 1))

            # ---- per-chunk LSTM matmul phases ----
            def gates_layer1(bc, t):
                """gates1 psum: recurrent + x (t=0) / one-hot table part."""
                gp = pg.tile([128, 8, 128], F32, tag="g1", name="g1")
                bs = bsl(bc)
                for g in range(8):
                    sl = slice(128 * g, 128 * (g + 1))
                    o = gp[:, g, :]
                    for k in range(2):
                        nc.tensor.matmul(o, w1hhh[:, k, sl], h1h[:, k, bs],
                                         start=(k == 0), stop=False)
                        nc.tensor.matmul(o, w1hhh[:, k, sl], h1l[:, k, bs],
                                         start=False, stop=False)
                        nc.tensor.matmul(o, w1hhl[:, k, sl], h1h[:, k, bs],
                                         start=False, stop=False)
                    if t == 0:
                        for k in range(2):
                            nc.tensor.matmul(o, w1ihh[:, k, sl], xh[:, k, bs],
                                             start=False, stop=False)
                            nc.tensor.matmul(o, w1ihh[:, k, sl], xl[:, k, bs],
                                             start=False, stop=False)
                            nc.tensor.matmul(o, w1ihl[:, k, sl], xh[:, k, bs],
                                             start=False, stop=(k == 1))
                    else:
                        nc.tensor.matmul(o, awh[:, sl], ohwT[:, bs],
                                         start=False, stop=False)
                        nc.tensor.matmul(o, awl[:, sl], ohwT[:, bs],
                                         start=False, stop=False)
                        nc.tensor.matmul(o, alh[:, sl], ohlT[:, bs],
                                         start=False, stop=False)
                        nc.tensor.matmul(o, all_[:, sl], ohlT[:, bs],
                                         start=False, stop=True)
                return gp

            def gates_layer2(bc):
                gp = pg.tile([128, 8, 128], F32, tag="g1", name="g2")
                bs = bsl(bc)
                for g in range(8):
                    sl = slice(128 * g, 128 * (g + 1))
                    o = gp[:, g, :]
                    for k in range(2):
                        nc.tensor.matmul(o, w2ihh[:, k, sl], h1h[:, k, bs],
                                         start=(k == 0), stop=False)
                        nc.tensor.matmul(o, w2ihh[:, k, sl], h1l[:, k, bs],
                                         start=False, stop=False)
                        nc.tensor.matmul(o, w2ihl[:, k, sl], h1h[:, k, bs],
                                         start=False, stop=False)
                    for k in range(2):
                        nc.tensor.matmul(o, w2hhh[:, k, sl], h2h[:, k, bs],
                                         start=False, stop=False)
                        nc.tensor.matmul(o, w2hhh[:, k, sl], h2l[:, k, bs],
                                         start=False, stop=False)
                        nc.tensor.matmul(o, w2hhl[:, k, sl], h2h[:, k, bs],
                                         start=False, stop=(k == 1))
                return gp

            def gate_acts(bc, gp, br):
                """sigmoid/tanh activations with per-slice gate biases."""
                si = wk.tile([128, 2, 128], F32, tag="si")
                sf = wk.tile([128, 2, 128], F32, tag="sf")
                tg = wk.tile([128, 2, 128], F32, tag="tg")
                so = wk.tile([128, 2, 128], F32, tag="so")
                for ch in range(2):
                    nc.scalar.activation(si[:, ch, :], gp[:, 0 + ch, :],
                                         AF.Sigmoid, bias=br[:, 0 + ch:1 + ch])
                    nc.scalar.activation(sf[:, ch, :], gp[:, 2 + ch, :],
                                         AF.Sigmoid, bias=br[:, 2 + ch:3 + ch])
                    nc.scalar.activation(tg[:, ch, :], gp[:, 4 + ch, :],
                                         AF.Tanh, bias=br[:, 4 + ch:5 + ch])
                    nc.scalar.activation(so[:, ch, :], gp[:, 6 + ch, :],
                                         AF.Sigmoid, bias=br[:, 6 + ch:7 + ch])
                return si, sf, tg, so

            def cell_update(bc, acts, cT, hT, hh, hl):
                si, sf, tg, so = acts
                bs = bsl(bc)
                csl = cT[:, :, bs]
                hsl = hT[:, :, bs]
                t1 = wk.tile([128, 2, 128], F32, tag="t1", bufs=1)
                t2 = wk.tile([128, 2, 128], F32, tag="t2", bufs=1)
                nc.vector.tensor_mul(t1[:], sf[:], csl)
                nc.vector.tensor_mul(t2[:], si[:], tg[:])
                nc.vector.tensor_add(csl, t1[:], t2[:])
                t3 = wk.tile([128, 2, 128], F32, tag="t3", bufs=1)
                nc.scalar.activation(t3[:], csl, AF.Tanh)
                nc.vector.tensor_mul(hsl, so[:], t3[:])
                nc.vector.tensor_copy(hh[:, :, bs], hsl)
                nc.vector.tensor_sub(hl[:, :, bs], hsl, hh[:, :, bs])

            # ---- fc3 + PSUM-direct scan for one chunk ----
            # 20 tiles: 19 x 512 + 1 x 272; scan groups of 2 tiles in one
            # flat 2-bank psum tile so indices stay affine (base 1024*gi)
            FTILES = [(i * 512, 512) for i in range(19)] + [(9728, 272)]

            def fc3_scan(bc):
                bs = bsl(bc)
                cand_v = wk.tile([128, 80], F32, tag="candv", name="candv")
                if not hasattr(fc3_scan, "ran"):
                    fc3_scan.ran = [False]
                cand_iu = wk.tile([128, 80], U32, tag="candiu", name="candiu")
                stats = [(h2h[:, 0, bs], w3h[:, 0, :]),
                         (h2h[:, 1, bs], w3h[:, 1, :]),
                         (h2l[:, 0, bs], w3h[:, 0, :]),
                         (h2l[:, 1, bs], w3h[:, 1, :]),
                         (h2h[:, 0, bs], w3l[:, 0, :]),
                         (h2h[:, 1, bs], w3l[:, 1, :])]
                for gi in range(NG):
                    pt = p3.tile([128, 1024], F32, tag="fc3p", name="fc3p")
                    spanw = 0
                    for ti in range(2):
                        n0, wdt = FTILES[2 * gi + ti]
                        o = pt[:, 512 * ti:512 * ti + wdt]
                        # seed PSUM with the fc3 bias triple, then accumulate
                        nc.tensor.matmul(o, one3[:], b3t[:, n0:n0 + wdt],
                                         start=True, stop=False)
                        for j, (stat, w) in enumerate(stats):
                            nc.tensor.matmul(o, stat, w[:, n0:n0 + wdt],
                                             start=False, stop=(j == 5))
                        spanw = 512 * ti + wdt
                    span = pt[:, 0:spanw]
                    if dbg and bc == 0 and gi == 0 and not fc3_scan.ran[0]:
                        lqg = wk.tile([128, 1024], F32, tag="lqg", name="lqg")
                        nc.vector.tensor_copy(lqg[:, 0:spanw], span)
                        nc.sync.dma_start(dbg_lq[:], lqg[:])
                    nc.vector.max(cand_v[:, 8 * gi:8 * gi + 8], span)
                    nc.vector.max_index(cand_iu[:, 8 * gi:8 * gi + 8],
                                        cand_v[:, 8 * gi:8 * gi + 8], span)
                cif = wk.tile([128, 80], F32, tag="cif", name="cif")
                nc.vector.tensor_copy(cif[:], cand_iu[:])
                cand_i = wk.tile([128, 80], F32, tag="candi", name="candi")
                nc.vector.tensor_add(cand_i[:], cif[:], iob80[:])
                if dbg and bc == 0 and not fc3_scan.ran[0]:
                    nc.sync.dma_start(dbg_cv[:], cand_v[:])
                    nc.sync.dma_start(dbg_ci[:], cand_i[:])
                    nc.sync.dma_start(dbg_h2[:], h2_t[:])
                    fc3_scan.ran[0] = True
                return cand_v, cand_i

            def merge_onehot(bc, t, cand_v, cand_i):
                """top-k merge, trajectory index write, one-hot build."""
                vm8 = wk.tile([128, 8], F32, tag="vm8", name="vm8")
                pm8 = wk.tile([128, 8], U32, tag="pm8", name="pm8")
                nc.vector.max(vm8[:], cand_v[:])
                nc.vector.max_index(pm8[:], vm8[:], cand_v[:])
                pmf = wk.tile([128, 8], F32, tag="pmf", name="pmf")
                nc.vector.tensor_copy(pmf[:], pm8[:])
                nk = 4 if t == 0 else 1
                qsel = wk.tile([128, 4], F32, tag="qsel", name="qsel")
                for kk in range(nk):
                    ohp = wk.tile([128, 80], F32, tag="ohp", name="ohp")
                    nc.vector.tensor_scalar(ohp[:], io80[:], pmf[:, kk:kk + 1],
                                            None, op0=ALU.is_equal)
                    tmq = wk.tile([128, 80], F32, tag="tmq", name="tmq")
                    nc.vector.tensor_mul(tmq[:], ohp[:], cand_i[:])
                    nc.vector.tensor_reduce(qsel[:, kk:kk + 1], tmq[:],
                                            axis=mybir.AxisListType.X,
                                            op=ALU.add)
                if t == 0:
                    nc.vector.tensor_copy(outi[:, bc, 0:4], qsel[:, 0:4])
                else:
                    nc.vector.tensor_copy(outi[:, bc, 4 + t - 1:5 + t - 1],
                                          qsel[:, 0:1])
                if t == delta - 1:
                    return None, None
                qf = qsel[:, 0:1]
                m_ge = wk.tile([128, 100], F32, tag="mge", name="mge", bufs=1)
                nc.vector.tensor_scalar(m_ge[:], io100[:], qf, None,
                                        op0=ALU.is_le)
                qm = wk.tile([128, 1], F32, tag="qm", name="qm")
                nc.vector.tensor_scalar(qm[:], qf, -100.0, None, op0=ALU.add)
                m_lt = wk.tile([128, 100], F32, tag="mlt", name="mlt", bufs=1)
                nc.vector.tensor_scalar(m_lt[:], io100[:], qm[:], None,
                                        op0=ALU.is_gt)
                ohw = wk.tile([128, 100], F32, tag="ohw", name="ohw", bufs=2)
                nc.vector.tensor_mul(ohw[:], m_ge[:], m_lt[:])
                tm = wk.tile([128, 100], F32, tag="tm", name="tm", bufs=1)
                nc.vector.tensor_mul(tm[:], ohw[:], io_f[:])
                fwf = wk.tile([128, 1], F32, tag="fwf", name="fwf")
                nc.vector.tensor_reduce(fwf[:], tm[:], axis=mybir.AxisListType.X,
                                        op=ALU.add)
                flf = wk.tile([128, 1], F32, tag="flf", name="flf")
                nc.vector.tensor_scalar(flf[:], fwf[:], -100.0, qf,
                                        op0=ALU.mult, op1=ALU.add)
                ohl = wk.tile([128, 100], F32, tag="ohl", name="ohl", bufs=2)
                nc.vector.tensor_scalar(ohl[:], io_f[:], flf[:], None,
                                        op0=ALU.is_equal)
                return ohw, ohl

            def trans_oh(bc, ohw, ohl):
                """transpose one-hots into [100, BS] bf16 table operands."""
                bs = bsl(bc)
                pw = p3.tile([128, 1024], F32, tag="fc3p", name="ptw")
                nc.tensor.transpose(pw[0:100, 0:128], ohw[:], ident[:])
                nc.vector.tensor_copy(ohwT[:, bs], pw[0:100, 0:128])
                nc.tensor.transpose(pw[0:100, 512:640], ohl[:], ident[:])
                nc.vector.tensor_copy(ohlT[:, bs], pw[0:100, 512:640])

            # ================= main loop (software-pipelined) =============
            # LSTM of step 0 (x-path prologue)
            for bc in range(2):
                gp = gates_layer1(bc, 0)
                if dbg and bc == 0:
                    g1c = wk.tile([128, 8, 128], F32, tag="g1c", name="g1c")
                    nc.vector.tensor_copy(g1c[:], gp[:])
                    nc.sync.dma_start(dbg_g1[:], g1c[:])
                a = gate_acts(bc, gp, b1r)
                cell_update(bc, a, c1_t, h1_t, h1h, h1l)
                if dbg and bc == 0:
                    nc.sync.dma_start(dbg_h1[:], h1_t[:])
                    nc.sync.dma_start(dbg_c1[:], c1_t[:])
            for bc in range(2):
                a = gate_acts(bc, gates_layer2(bc), b2r)
                cell_update(bc, a, c2_t, h2_t, h2h, h2l)

            for t in range(delta):
                last = (t == delta - 1)
                cv0, ci0 = fc3_scan(0)
                m0 = merge_onehot(0, t, cv0, ci0)
                cv1, ci1 = fc3_scan(1)
                if not last:
                    trans_oh(0, m0[0], m0[1])
                    gp1a = gates_layer1(0, t + 1)
                    aa = gate_acts(0, gp1a, b1rf)
                m1 = merge_onehot(1, t, cv1, ci1)
                if not last:
                    trans_oh(1, m1[0], m1[1])
                    gp1b = gates_layer1(1, t + 1)
                    ab = gate_acts(1, gp1b, b1rf)
                    cell_update(0, aa, c1_t, h1_t, h1h, h1l)
                    cell_update(1, ab, c1_t, h1_t, h1h, h1l)
                    a2a = gate_acts(0, gates_layer2(0), b2r)
                    cell_update(0, a2a, c2_t, h2_t, h2h, h2l)
                    a2b = gate_acts(1, gates_layer2(1), b2r)
                    cell_update(1, a2b, c2_t, h2_t, h2h, h2l)

            for bc in range(2):
                nc.sync.dma_start(idx_out[bc], outi[:, bc, :])
    nc.finalize()
    return nc


def _prep_shared(inputs):
    f32, f64 = np.float32, np.float64
    bf = ml_dtypes.bfloat16

    def split(a):
        ah = a.astype(bf)
        al = (a.astype(f32) - ah.astype(f32)).astype(bf)
        return ah, al

    def fmT(w):  # [out, in] -> lhsT chunks [2, 128, out]
        wt = np.ascontiguousarray(w.T.astype(f32))
        return wt.reshape(2, 128, wt.shape[1])

    W = {k: np.asarray(v) for k, v in inputs.items()}
    fc1, fc2, fc3 = (W['fc1_W'].astype(f64), W['fc2_W'].astype(f64),
                     W['fc3_W'].astype(f64))
    W3f = (fc3 @ fc2 @ fc1).astype(f32)                       # [Q, 256]
    b3f = (W['fc3_b'].astype(f64) + fc3 @ W['fc2_b'].astype(f64)
           + (fc3 @ fc2) @ W['fc1_b'].astype(f64)).astype(f32)
    Aw = (W['lstm1_Wih'][:, :128].astype(f64)
          @ W['fcqw_W'].astype(f64)).astype(f32)              # [1024, 100]
    Al = (W['lstm1_Wih'][:, 128:].astype(f64)
          @ W['fcql_W'].astype(f64)).astype(f32)
    embb = np.concatenate([W['fcqw_b'], W['fcql_b']]).astype(f64)
    b1f = (W['lstm1_b'].astype(f64)
           + W['lstm1_Wih'].astype(f64) @ embb).astype(f32)

    shared = {}
    for name, w in (("w1ih", W['lstm1_Wih']), ("w1hh", W['lstm1_Whh']),
                    ("w2ih", W['lstm2_Wih']), ("w2hh", W['lstm2_Whh'])):
        h_, l_ = split(fmT(w))
        shared[name + "Th"] = h_
        shared[name + "Tl"] = l_
    shared["w3Th"], shared["w3Tl"] = split(fmT(W3f))
    shared["awTh"], shared["awTl"] = split(np.ascontiguousarray(Aw.T))
    shared["alTh"], shared["alTl"] = split(np.ascontiguousarray(Al.T))
    shared["b1r"] = W['lstm1_b'].astype(f32).reshape(8, 128).T.copy()
    shared["b1rf"] = b1f.reshape(8, 128).T.copy()
    shared["b2r"] = W['lstm2_b'].astype(f32).reshape(8, 128).T.copy()
    # fc3 bias as 3 bf16 terms (seeded into PSUM via a K=3 ones matmul)
    b3a = b3f.astype(bf)
    r1 = (b3f - b3a.astype(f32)).astype(f32)
    b3b = r1.astype(bf)
    b3c = (r1 - b3b.astype(f32)).astype(bf)
    shared["b3t"] = np.ascontiguousarray(np.stack([b3a, b3b, b3c]))
    # per-candidate-slot global index base: slot 8g+k -> 1024g
    iob = np.repeat(np.arange(NG, dtype=f32) * 1024.0, 8)
    shared["iob80"] = np.ascontiguousarray(np.broadcast_to(iob, (128, 80)))
    return shared


def _per_core(inputs, c):
    f32 = np.float32
    bf = ml_dtypes.bfloat16
    sl = slice(c * BS, (c + 1) * BS)

    def fmT(a):  # [BS, 256] -> [2, 128, BS]
        return np.ascontiguousarray(a.T.astype(f32)).reshape(2, 128, BS)

    def split(a):
        ah = a.astype(bf)
        al = (a - ah.astype(f32)).astype(bf)
        return ah, al

    x = fmT(np.asarray(inputs["x"])[sl, 0, :])
    h1 = fmT(np.asarray(inputs["h1"])[0, sl])
    h2 = fmT(np.asarray(inputs["h2"])[0, sl])
    xh, xl = split(x)
    h1h, h1l = split(h1)
    h2h, h2l = split(h2)
    return {
        "xh": xh, "xl": xl,
        "c1_fm": fmT(np.asarray(inputs["c1"])[0, sl]),
        "c2_fm": fmT(np.asarray(inputs["c2"])[0, sl]),
        "h1h": h1h, "h1l": h1l, "h2h": h2h, "h2l": h2l,
    }


def kernel(**inputs):
    key = "nc"
    if key not in _CACHE:
        _CACHE[key] = _build_nc()
    nc = _CACHE[key]

    shared = _prep_shared(inputs)
    in_maps = []
    for c in range(NCORES):
        m = dict(shared)
        m.update(_per_core(inputs, c))
        in_maps.append(m)

    from concourse.bass_utils import run_bass_kernel_spmd
    res = run_bass_kernel_spmd(nc, in_maps, list(range(NCORES)))
    return assemble(res.results)


def assemble(results):
    traj = np.zeros((B, DELTA, K4, 2), np.float32)
    for c, r in enumerate(results):
        idx = r["idx_out"].reshape(2, 128, 20).astype(np.int64)
        for bc in range(2):
            rows = slice(c * BS + bc * 128, c * BS + (bc + 1) * 128)
            top4 = idx[bc, :, 0:4]
            traj[rows, 0, :, 0] = (top4 % QL).astype(np.float32)
            traj[rows, 0, :, 1] = (top4 // QL).astype(np.float32)
            greedy = idx[bc, :, 4:4 + DELTA - 1]
            traj[rows, 1:, 0, 0] = (greedy % QL).astype(np.float32)
            traj[rows, 1:, 0, 1] = (greedy // QL).astype(np.float32)
    return traj
